# revision 3
# baseline (speedup 1.0000x reference)
"""DepthAttentionResidual Trainium2 kernel.

Computation (see reference):
    ms      = mean(history^2, axis=-1)                      # [S,B,T]
    logits  = dot(query*rms_weight, history) * rsqrt(ms+eps)
    w       = softmax(logits, axis=S)
    out     = sum_s w[s] * history[s]                        # [B,T,D]

Sharding: data-parallel over (B=4) x (T halves) = 8 cores. Each core gets
hist [S=16, Tc=1024, D=1024] (64 MiB) and produces out [1024, 1024].

Per-core layout: partition p = s*8 + t' (S=16 depths x 8 t-blocks), D on
the free axis. A supertile is 128 t; slice g holds t_local = t'*16 + g.
J=8 consecutive t rows ride one DMA descriptor (32 KiB contiguous on
both sides), so a supertile loads with TWO dma_starts of 4 MiB each --
the 4 KiB-descriptor regime measured only ~67% DMA-engine efficiency
(~120 ns fixed cost per packet); 32 KiB descriptors run ~92%.
  - sum(h^2) over D: ScalarE activation(Square, accum_out), all 16
  - dot(q*w, h) over D: VectorE affine_mul_reduce, all 16
  - rstd via exp(-0.5*ln(ms+eps)) so ScalarE only ever needs the
    natural_log_exp_and_others table (Square/Ln/Exp/Copy): the baseline's
    Sqrt forced 2 ACT_TABLE_LOADs (2.6 us) per supertile
  - softmax over S: sum over s-partition-subgroups via 0/1-mask fp32
    matmuls (exact); reciprocal on VectorE
  - depth mix: per D-half, 16 accumulating fp32r matmuls (1 cycle/row)
    with block-expanded masked weights built on GpSimd from ew = e/sumexp
    (premultiplied on VectorE); PSUM -> SBUF drains on GpSimd
Input DMAs ride the SP HWDGE ring alone; output + mask constants ride
the GpSimd SWDGE ring; query/rms_weight are broadcast on-chip via a
1-partition ones matmul instead of a 256-packet broadcast DMA.
The last supertile is split 64/64 to shorten the serial tail. fp32r
rounds operands to ~13 mantissa bits -> ~2e-4 relative output error.

Reads history exactly once (~68 MiB DMA per core): DMA-bound at a
~196 us 16-engine descriptor floor; ScalarE/VectorE each ~175 us.
"""
import numpy as np

import concourse.bass as bass
import concourse.bacc as bacc
import concourse.tile as tile
from concourse import mybir
from concourse import bass_utils

N_CORES = 8
S = 16
B = 4
T = 2048
D = 1024
EPS = 1e-5

TC = T // 2          # t positions per core
TG = 8               # t-blocks per partition set (S * TG = 128 partitions)
GROUPS = 16          # stat slices per supertile (one t per partition each)
J = 8                # consecutive t rows per DMA descriptor (J*4KiB each)
TS = TG * GROUPS     # t per supertile = 128
N_SUPER = TC // TS   # supertiles per core = 8
F32 = mybir.dt.float32
F32R = mybir.dt.float32r


def _build_program():
    nc = bacc.Bacc("TRN2", target_bir_lowering=False, debug=False,
                   enable_asserts=True, num_devices=N_CORES)

    hist = nc.dram_tensor("hist", [S, TC, D], F32R, kind="ExternalInput").ap()
    query = nc.dram_tensor("query", [D], F32, kind="ExternalInput").ap()
    rmsw = nc.dram_tensor("rms_weight", [D], F32, kind="ExternalInput").ap()
    mask_d = nc.dram_tensor("mask", [128, TG], F32, kind="ExternalInput").ap()
    maskt_d = nc.dram_tensor("maskT", [TG, 128], F32, kind="ExternalInput").ap()
    maskf_d = nc.dram_tensor("maskF", [128, GROUPS, 128], F32,
                             kind="ExternalInput").ap()
    maskf8_d = nc.dram_tensor("maskF8", [128, GROUPS // 2, 128 // 2], F32,
                              kind="ExternalInput").ap()
    out = nc.dram_tensor("out", [TC, D], F32, kind="ExternalOutput").ap()

    with tile.TileContext(nc) as tc:
        with (
            tc.tile_pool(name="singles", bufs=1) as singles,
            tc.tile_pool(name="hsup", bufs=2) as hpool,
            tc.tile_pool(name="stats", bufs=2) as stats,
            tc.tile_pool(name="w2", bufs=3) as w2pool,
            tc.tile_pool(name="outp", bufs=2) as outpool,
            tc.tile_pool(name="ps_stats", bufs=2, space="PSUM") as ps_stats,
            tc.tile_pool(name="ps_mix", bufs=2, space="PSUM") as ps_mix,
        ):
            # ---- constants ------------------------------------------------
            qw = singles.tile([128, D], F32)
            q1 = singles.tile([1, D], F32)
            w1 = singles.tile([1, D], F32)
            ones = singles.tile([1, 128], F32)
            mask = singles.tile([128, TG], F32)
            maskT = singles.tile([TG, 128], F32)
            maskF = singles.tile([128, GROUPS, 128], F32)
            maskF8 = singles.tile([128, GROUPS // 2, 128 // 2], F32)
            epst = singles.tile([128, 1], F32)
            dummy_a = singles.tile([128, 1], F32)
            dummy_v = singles.tile([128, 1], F32)

            def emit_init():
                # constants ride the GpSimd SWDGE queue so they never
                # contend with the history stream on the SP HWDGE ring
                nc.gpsimd.dma_start(
                    out=q1[:],
                    in_=bass.AP(tensor=query.tensor, offset=0,
                                ap=[[0, 1], [1, D]]),
                )
                nc.gpsimd.dma_start(
                    out=w1[:],
                    in_=bass.AP(tensor=rmsw.tensor, offset=0,
                                ap=[[0, 1], [1, D]]),
                )
                nc.gpsimd.dma_start(out=mask[:], in_=mask_d)
                nc.gpsimd.dma_start(out=maskT[:], in_=maskt_d)
                nc.gpsimd.dma_start(out=maskF[:], in_=maskf_d)
                nc.gpsimd.dma_start(out=maskF8[:], in_=maskf8_d)
                nc.vector.memset(epst[:], EPS)
                nc.vector.memset(ones[:], 1.0)
                nc.vector.tensor_mul(q1[:], q1[:], w1[:])  # query * rms_weight
                # broadcast q1 to all 128 partitions: ones^T @ q1 via PE
                for c in range(2):
                    bc = ps_mix.tile([128, 512], F32, tag="m", name=f"bc{c}")
                    nc.tensor.matmul(out=bc[:], lhsT=ones[:],
                                     rhs=q1[:, c * 512:(c + 1) * 512],
                                     start=True, stop=True)
                    nc.scalar.activation(
                        out=qw[:, c * 512:(c + 1) * 512], in_=bc[:],
                        func=mybir.ActivationFunctionType.Copy,
                    )

            # ---- main loop over supertiles --------------------------------
            # last 128-t supertile is split in two 64-t halves to shorten
            # the serial tail after the final DMA
            emit_init()
            schedule = [(k * TS, GROUPS) for k in range(N_SUPER - 1)]
            schedule += [((N_SUPER - 1) * TS, GROUPS // 2),
                         ((N_SUPER - 1) * TS + TS // 2, GROUPS // 2)]
            for k, (t0, groups) in enumerate(schedule):
                ts_k = TG * groups
                ndma = groups // J

                # load [S, ts_k, D] as partitions (s, t') x free (g, d)
                # where t_local = t' * groups + g: one descriptor per
                # partition per dma_start, J*4KiB contiguous on both sides
                hsupA = hpool.tile([128, J, D], F32R, tag="hsupA",
                                   name="hsupA", bufs=3)
                hsupB = (hpool.tile([128, J, D], F32R, tag="hsupB",
                                    name="hsupB") if ndma > 1 else None)

                def hslice(g):
                    gd, j = g // J, g % J
                    return (hsupA if gd == 0 else hsupB)[:, j, :]

                srcv = hist[:, t0:t0 + ts_k, :].rearrange(
                    "s (t gd j) d -> s t gd (j d)", t=TG, gd=ndma)
                for gd in range(ndma):
                    tile_ = hsupA if gd == 0 else hsupB
                    nc.sync.dma_start(
                        out=tile_[:].rearrange("p j d -> p (j d)"),
                        in_=srcv[:, :, gd, :])

                ss = stats.tile([128, groups], F32, tag="ss")
                dot = stats.tile([128, groups], F32, tag="dot")
                for g in range(groups):
                    h_g = hslice(g).bitcast(F32)
                    nc.scalar.activation(
                        out=dummy_a.broadcast_to([128, D]),
                        in_=h_g,
                        func=mybir.ActivationFunctionType.Square,
                        accum_out=ss[:, g:g + 1],
                    )
                    nc.vector.affine_mul_reduce(
                        out=dummy_v.broadcast_to([128, D]),
                        accum_out=dot[:, g:g + 1],
                        in0=h_g,
                        in1=qw[:],
                        scale=1.0,
                        bias=0.0,
                    )

                # rstd = exp(-0.5*ln(ss/D + eps)); logits = dot * rstd
                # (Ln/Exp/Square share one activation table; Sqrt does not)
                lms = stats.tile([128, groups], F32, tag="lms")
                nc.scalar.activation(
                    out=lms[:], in_=ss[:],
                    func=mybir.ActivationFunctionType.Ln,
                    bias=epst[:], scale=1.0 / D,
                )
                rstd = stats.tile([128, groups], F32, tag="rstd")
                nc.scalar.activation(
                    out=rstd[:], in_=lms[:],
                    func=mybir.ActivationFunctionType.Exp,
                    scale=-0.5,
                )
                logit = stats.tile([128, groups], F32, tag="logit")
                nc.vector.tensor_mul(logit[:], dot[:], rstd[:])
                e = stats.tile([128, groups], F32, tag="e")
                nc.scalar.activation(
                    out=e[:], in_=logit[:],
                    func=mybir.ActivationFunctionType.Exp,
                )

                # sumexp over s: [8t', G] = mask^T @ e (exact fp32 matmul)
                se_ps = ps_stats.tile([TG, groups], F32, tag="se")
                nc.tensor.matmul(out=se_ps[:], lhsT=mask[:], rhs=e[:],
                                 start=True, stop=True)
                rse = stats.tile([TG, groups], F32, tag="rse")
                nc.vector.reciprocal(out=rse[:], in_=se_ps[:])
                # broadcast rse back to (s,t') partitions: maskT^T @ rse
                rseb_ps = ps_stats.tile([128, groups], F32, tag="rseb")
                nc.tensor.matmul(out=rseb_ps[:], lhsT=maskT[:], rhs=rse[:],
                                 start=True, stop=True)
                # ew = e / sumexp: premultiplied so the GpSimd w2 build
                # needs a single scalar multiply
                ew = stats.tile([128, groups], F32, tag="ew")
                nc.vector.tensor_mul(ew[:], e[:], rseb_ps[:])

                # depth mix: accumulate 16 masked-weight matmuls per D chunk
                m_ps = [ps_mix.tile([TG * groups, 512], F32, tag="m", name=f"m{c}")
                        for c in range(2)]
                for g in range(groups):
                    w2 = w2pool.tile([128, TG * groups], F32R, tag="w2")
                    nc.gpsimd.tensor_scalar(
                        out=w2[:],
                        in0=(maskF[:, g, :] if groups == GROUPS
                             else maskF8[:, g, :]),
                        scalar1=ew[:, g:g + 1],
                        scalar2=None,
                        op0=mybir.AluOpType.mult,
                    )
                    for c in range(2):
                        nc.tensor.matmul(
                            out=m_ps[c][:],
                            lhsT=w2[:],
                            rhs=hslice(g)[:, c * 512:(c + 1) * 512],
                            start=(g == 0),
                            stop=(g == groups - 1),
                        )

                # GPSIMD cannot access PSUM: drain one half on ScalarE
                # (Copy lives in the same act table) and one on VectorE
                ot = outpool.tile([TG * groups, D], F32, tag="ot")
                nc.scalar.activation(
                    out=ot[:, 0:512], in_=m_ps[0][:],
                    func=mybir.ActivationFunctionType.Copy,
                )
                nc.vector.tensor_copy(out=ot[:, 512:1024], in_=m_ps[1][:])
                nc.gpsimd.dma_start(out=out[t0:t0 + ts_k, :], in_=ot[:])

    nc.compile()
    return nc


_NC = None


def _get_program():
    global _NC
    if _NC is None:
        _NC = _build_program()
    return _NC


def _make_masks():
    # partition p = s*TG + t'; group slice g holds t_local = t'*GROUPS + g
    p = np.arange(128)
    mask = (p[:, None] % TG == np.arange(TG)[None, :]).astype(np.float32)
    maskF = np.zeros((128, GROUPS, 128), np.float32)
    for g in range(GROUPS):
        maskF[p, g, (p % TG) * GROUPS + g] = 1.0
    maskF8 = np.zeros((128, GROUPS // 2, 64), np.float32)
    for g in range(GROUPS // 2):
        maskF8[p, g, (p % TG) * (GROUPS // 2) + g] = 1.0
    return mask, np.ascontiguousarray(mask.T), maskF, maskF8


def kernel(history, query, rms_weight):
    history = np.asarray(history, dtype=np.float32)
    query = np.asarray(query, dtype=np.float32)
    rms_weight = np.asarray(rms_weight, dtype=np.float32)
    assert history.shape == (S, B, T, D), history.shape

    nc = _get_program()
    mask, maskT, maskF, maskF8 = _make_masks()

    in_maps = []
    for c in range(N_CORES):
        b, h = c // 2, c % 2
        shard = np.ascontiguousarray(history[:, b, h * TC:(h + 1) * TC, :])
        in_maps.append({
            "hist": shard,
            "query": query,
            "rms_weight": rms_weight,
            "mask": mask,
            "maskT": maskT,
            "maskF": maskF,
            "maskF8": maskF8,
        })

    res = bass_utils.run_bass_kernel_spmd(nc, in_maps, list(range(N_CORES)))

    out = np.empty((B, T, D), dtype=np.float32)
    for c in range(N_CORES):
        b, h = c // 2, c % 2
        out[b, h * TC:(h + 1) * TC, :] = res.results[c]["out"]
    return out


# revision 7
# speedup vs baseline: 1.0401x; 1.0401x over previous
"""DepthAttentionResidual Trainium2 kernel.

Computation (see reference):
    ms      = mean(history^2, axis=-1)                      # [S,B,T]
    logits  = dot(query*rms_weight, history) * rsqrt(ms+eps)
    w       = softmax(logits, axis=S)
    out     = sum_s w[s] * history[s]                        # [B,T,D]

Sharding: data-parallel over (B=4) x (T halves) = 8 cores. Each core gets
hist [S=16, Tc=1024, D=1024] (64 MiB) and produces out [1024, 1024].

The per-core DMA subsystem measures ~235 GB/s regardless of descriptor
size (16 engines x ~14 B/ns, latency-bound), so the kernel is pinned at
~300 us of DMA wall time for its 68.4 MiB; everything else hides under
that. Engine budget per supertile (~21-23 us each, under the ~24 us DMA
period): ScalarE 16 Square+accum passes, VectorE 16 affine_mul_reduce
dot passes + softmax smalls, PE the masked-weight depth mix, GpSimd the
w2 build.

Per-core layout: partition p = s*8 + t' (S=16 depths x 8 t-blocks), D on
the free axis. A supertile is 128 t; slice g holds t_local = t'*16 + g.
  - rstd = rsqrt(ms+eps) via the int32 magic-constant seed + 2 Newton
    steps on VectorE: no Sqrt activation, so ScalarE only ever uses the
    exp_and_others table (Square/Exp/Copy) -> one ACT_TABLE_LOAD total
    (the baseline's Sqrt cost 2 x 1.28 us table swaps per supertile)
  - softmax over S: sum over s-partition-subgroups via 0/1-mask fp32
    matmuls (exact); reciprocal on VectorE
  - depth mix: per D-half, 16 accumulating fp32r matmuls (1 cycle/row)
    with block-expanded masked weights built on GpSimd (two-scalar
    tensor_scalar: e and 1/sumexp); PSUM drains split Scalar/Vector
    (GpSimd cannot access PSUM; SWDGE descriptor gen on the Q7s is also
    ~10x the spec cost, so all DMA stays on the SP/Act HWDGE rings)
  - query*rms_weight broadcast to 128 partitions on-chip via a
    1-partition ones matmul instead of a 256-packet broadcast DMA
Input DMAs alternate SP/Act HWDGE rings; output + masks ride Act.
The last supertile is split 64/64 to shorten the serial tail. fp32r
rounds operands to ~13 mantissa bits -> ~2e-4 relative output error.
"""
import numpy as np

import concourse.bass as bass
import concourse.bacc as bacc
import concourse.tile as tile
from concourse import mybir
from concourse import bass_utils

N_CORES = 8
S = 16
B = 4
T = 2048
D = 1024
EPS = 1e-5

TC = T // 2          # t positions per core
TG = 8               # t-blocks per partition set (S * TG = 128 partitions)
GROUPS = 16          # stat slices per supertile (one t per partition each)
J = 1                # consecutive t rows per DMA descriptor (J*4KiB each)
TS = TG * GROUPS     # t per supertile = 128
N_SUPER = TC // TS   # supertiles per core = 8
F32 = mybir.dt.float32
I32 = mybir.dt.int32
F32R = mybir.dt.float32r
RSQRT_MAGIC = 0x5F3759DF


def _build_program():
    nc = bacc.Bacc("TRN2", target_bir_lowering=False, debug=False,
                   enable_asserts=True, num_devices=N_CORES)

    hist = nc.dram_tensor("hist", [S, TC, D], F32R, kind="ExternalInput").ap()
    query = nc.dram_tensor("query", [D], F32, kind="ExternalInput").ap()
    rmsw = nc.dram_tensor("rms_weight", [D], F32, kind="ExternalInput").ap()
    mask_d = nc.dram_tensor("mask", [128, TG], F32, kind="ExternalInput").ap()
    maskt_d = nc.dram_tensor("maskT", [TG, 128], F32, kind="ExternalInput").ap()
    maskf_d = nc.dram_tensor("maskF", [128, GROUPS, 128], F32,
                             kind="ExternalInput").ap()
    maskf8_d = nc.dram_tensor("maskF8", [128, GROUPS // 2, 128 // 2], F32,
                              kind="ExternalInput").ap()
    out = nc.dram_tensor("out", [TC, D], F32, kind="ExternalOutput").ap()

    with tile.TileContext(nc) as tc:
        with (
            tc.tile_pool(name="singles", bufs=1) as singles,
            tc.tile_pool(name="hsup", bufs=2) as hpool,
            tc.tile_pool(name="stats", bufs=2) as stats,
            tc.tile_pool(name="w2", bufs=3) as w2pool,
            tc.tile_pool(name="outp", bufs=2) as outpool,
            tc.tile_pool(name="ps_stats", bufs=2, space="PSUM") as ps_stats,
            tc.tile_pool(name="ps_mix", bufs=2, space="PSUM") as ps_mix,
        ):
            # ---- constants ------------------------------------------------
            qw = singles.tile([128, D], F32)
            q1 = singles.tile([1, D], F32)
            w1 = singles.tile([1, D], F32)
            ones = singles.tile([1, 128], F32)
            mask = singles.tile([128, TG], F32)
            maskT = singles.tile([TG, 128], F32)
            maskF = singles.tile([128, GROUPS, 128], F32)
            maskF8 = singles.tile([128, GROUPS // 2, 128 // 2], F32)
            dummy_a = singles.tile([128, 1], F32)
            dummy_v = singles.tile([128, 1], F32)

            def emit_init():
                # constants ride the Act HWDGE ring behind nothing; the
                # input stream owns SP from t=0
                nc.scalar.dma_start(
                    out=q1[:],
                    in_=bass.AP(tensor=query.tensor, offset=0,
                                ap=[[0, 1], [1, D]]),
                )
                nc.scalar.dma_start(
                    out=w1[:],
                    in_=bass.AP(tensor=rmsw.tensor, offset=0,
                                ap=[[0, 1], [1, D]]),
                )
                nc.scalar.dma_start(out=mask[:], in_=mask_d)
                nc.scalar.dma_start(out=maskT[:], in_=maskt_d)
                nc.scalar.dma_start(out=maskF[:], in_=maskf_d)
                nc.scalar.dma_start(out=maskF8[:], in_=maskf8_d)
                nc.vector.memset(ones[:], 1.0)
                nc.vector.tensor_mul(q1[:], q1[:], w1[:])  # query * rms_weight
                # broadcast q1 to all 128 partitions: ones^T @ q1 via PE
                for c in range(2):
                    bc = ps_mix.tile([128, 512], F32, tag="m", name=f"bc{c}")
                    nc.tensor.matmul(out=bc[:], lhsT=ones[:],
                                     rhs=q1[:, c * 512:(c + 1) * 512],
                                     start=True, stop=True)
                    nc.scalar.activation(
                        out=qw[:, c * 512:(c + 1) * 512], in_=bc[:],
                        func=mybir.ActivationFunctionType.Copy,
                    )

            # ---- main loop over supertiles --------------------------------
            # last 128-t supertile is split in two 64-t halves to shorten
            # the serial tail after the final DMA
            emit_init()
            schedule = [(k * TS, GROUPS) for k in range(N_SUPER - 1)]
            schedule += [((N_SUPER - 1) * TS, GROUPS // 2),
                         ((N_SUPER - 1) * TS + TS // 2, GROUPS // 2)]
            dma_i = 0
            for k, (t0, groups) in enumerate(schedule):
                ts_k = TG * groups
                ndma = groups // J
                half = ndma // 2

                # load [S, ts_k, D] as partitions (s, t') x free (g, d)
                # where t_local = t' * groups + g; one descriptor per
                # partition per dma_start, J*4KiB contiguous on both sides
                hsupA = hpool.tile([128, half, J, D], F32R, tag="hsupA",
                                   name="hsupA", bufs=3)
                hsupB = hpool.tile([128, ndma - half, J, D], F32R,
                                   tag="hsupB", name="hsupB")

                def hslice(g):
                    gd, j = g // J, g % J
                    tile_ = hsupA if gd < half else hsupB
                    return tile_[:, gd - half if gd >= half else gd, j, :]

                srcv = hist[:, t0:t0 + ts_k, :].rearrange(
                    "s (t gd j) d -> s t gd (j d)", t=TG, gd=ndma)
                for gd in range(ndma):
                    tile_ = hsupA if gd < half else hsupB
                    eng = nc.sync if dma_i % 2 == 0 else nc.scalar
                    dma_i += 1
                    eng.dma_start(
                        out=tile_[:, gd - half if gd >= half else gd, :, :]
                        .rearrange("p j d -> p (j d)"),
                        in_=srcv[:, :, gd, :])

                # ScalarE also issues half the input DMAs (~5 us/supertile
                # of SEQ time), so VectorE picks up 2 of the 16 sumsq
                ss = stats.tile([128, groups], F32, tag="ss")
                dot = stats.tile([128, groups], F32, tag="dot")
                n_vec_sq = 2 if groups == GROUPS else 1
                for g in range(groups):
                    h_g = hslice(g).bitcast(F32)
                    if g < groups - n_vec_sq:
                        nc.scalar.activation(
                            out=dummy_a.broadcast_to([128, D]),
                            in_=h_g,
                            func=mybir.ActivationFunctionType.Square,
                            accum_out=ss[:, g:g + 1],
                        )
                    else:
                        nc.vector.affine_mul_reduce(
                            out=dummy_v.broadcast_to([128, D]),
                            accum_out=ss[:, g:g + 1],
                            in0=h_g, in1=h_g, scale=1.0, bias=0.0,
                        )
                    nc.vector.affine_mul_reduce(
                        out=dummy_v.broadcast_to([128, D]),
                        accum_out=dot[:, g:g + 1],
                        in0=h_g,
                        in1=qw[:],
                        scale=1.0,
                        bias=0.0,
                    )

                # rstd = rsqrt(ss/D + eps) via magic-constant + 2 Newton
                # steps, entirely on VectorE (no activation table needed)
                v = stats.tile([128, groups], F32, tag="v")
                nc.vector.tensor_scalar(
                    out=v[:], in0=ss[:], scalar1=1.0 / D, scalar2=EPS,
                    op0=mybir.AluOpType.mult, op1=mybir.AluOpType.add)
                y = stats.tile([128, groups], F32, tag="y")
                nc.vector.tensor_scalar(
                    out=y[:].bitcast(I32), in0=v[:].bitcast(I32),
                    scalar1=1, scalar2=None,
                    op0=mybir.AluOpType.logical_shift_right)
                nc.vector.tensor_scalar(
                    out=y[:].bitcast(I32), in0=y[:].bitcast(I32),
                    scalar1=-1, scalar2=RSQRT_MAGIC,
                    op0=mybir.AluOpType.mult, op1=mybir.AluOpType.add)
                t1 = stats.tile([128, groups], F32, tag="t1")
                t2 = stats.tile([128, groups], F32, tag="t2")
                for _ in range(2):  # y *= 1.5 - 0.5*v*y^2
                    nc.vector.tensor_mul(t1[:], y[:], y[:])
                    nc.vector.tensor_mul(t2[:], v[:], t1[:])
                    nc.vector.tensor_scalar(
                        out=t2[:], in0=t2[:], scalar1=-0.5, scalar2=1.5,
                        op0=mybir.AluOpType.mult, op1=mybir.AluOpType.add)
                    nc.vector.tensor_mul(y[:], y[:], t2[:])

                logit = stats.tile([128, groups], F32, tag="logit")
                nc.vector.tensor_mul(logit[:], dot[:], y[:])
                e = stats.tile([128, groups], F32, tag="e")
                nc.scalar.activation(
                    out=e[:], in_=logit[:],
                    func=mybir.ActivationFunctionType.Exp,
                )

                # sumexp over s: [8t', G] = mask^T @ e (exact fp32 matmul)
                se_ps = ps_stats.tile([TG, groups], F32, tag="se")
                nc.tensor.matmul(out=se_ps[:], lhsT=mask[:], rhs=e[:],
                                 start=True, stop=True)
                rse = stats.tile([TG, groups], F32, tag="rse")
                nc.vector.reciprocal(out=rse[:], in_=se_ps[:])
                # broadcast rse back to (s,t') partitions: maskT^T @ rse
                rseb_ps = ps_stats.tile([128, groups], F32, tag="rseb")
                nc.tensor.matmul(out=rseb_ps[:], lhsT=maskT[:], rhs=rse[:],
                                 start=True, stop=True)
                rseb = stats.tile([128, groups], F32, tag="rsebs")
                nc.vector.tensor_copy(out=rseb[:], in_=rseb_ps[:])

                # depth mix: accumulate 16 masked-weight matmuls per D chunk
                m_ps = [ps_mix.tile([TG * groups, 512], F32, tag="m", name=f"m{c}")
                        for c in range(2)]
                for g in range(groups):
                    w2 = w2pool.tile([128, TG * groups], F32R, tag="w2")
                    nc.gpsimd.tensor_scalar(
                        out=w2[:],
                        in0=(maskF[:, g, :] if groups == GROUPS
                             else maskF8[:, g, :]),
                        scalar1=e[:, g:g + 1],
                        scalar2=rseb[:, g:g + 1],
                        op0=mybir.AluOpType.mult,
                        op1=mybir.AluOpType.mult,
                    )
                    for c in range(2):
                        nc.tensor.matmul(
                            out=m_ps[c][:],
                            lhsT=w2[:],
                            rhs=hslice(g)[:, c * 512:(c + 1) * 512],
                            start=(g == 0),
                            stop=(g == groups - 1),
                        )

                # PSUM -> SBUF: GpSimd cannot touch PSUM; split Sc/Ve
                ot = outpool.tile([TG * groups, D], F32, tag="ot")
                nc.scalar.activation(
                    out=ot[:, 0:512], in_=m_ps[0][:],
                    func=mybir.ActivationFunctionType.Copy,
                )
                nc.vector.tensor_copy(out=ot[:, 512:1024], in_=m_ps[1][:])
                eng = nc.sync if dma_i % 2 == 0 else nc.scalar
                dma_i += 1
                eng.dma_start(out=out[t0:t0 + ts_k, :], in_=ot[:])

    nc.compile()
    return nc


_NC = None


def _get_program():
    global _NC
    if _NC is None:
        _NC = _build_program()
    return _NC


def _make_masks():
    # partition p = s*TG + t'; group slice g holds t_local = t'*GROUPS + g
    p = np.arange(128)
    mask = (p[:, None] % TG == np.arange(TG)[None, :]).astype(np.float32)
    maskF = np.zeros((128, GROUPS, 128), np.float32)
    for g in range(GROUPS):
        maskF[p, g, (p % TG) * GROUPS + g] = 1.0
    maskF8 = np.zeros((128, GROUPS // 2, 64), np.float32)
    for g in range(GROUPS // 2):
        maskF8[p, g, (p % TG) * (GROUPS // 2) + g] = 1.0
    return mask, np.ascontiguousarray(mask.T), maskF, maskF8


def kernel(history, query, rms_weight):
    history = np.asarray(history, dtype=np.float32)
    query = np.asarray(query, dtype=np.float32)
    rms_weight = np.asarray(rms_weight, dtype=np.float32)
    assert history.shape == (S, B, T, D), history.shape

    nc = _get_program()
    mask, maskT, maskF, maskF8 = _make_masks()

    in_maps = []
    for c in range(N_CORES):
        b, h = c // 2, c % 2
        shard = np.ascontiguousarray(history[:, b, h * TC:(h + 1) * TC, :])
        in_maps.append({
            "hist": shard,
            "query": query,
            "rms_weight": rms_weight,
            "mask": mask,
            "maskT": maskT,
            "maskF": maskF,
            "maskF8": maskF8,
        })

    res = bass_utils.run_bass_kernel_spmd(nc, in_maps, list(range(N_CORES)))

    out = np.empty((B, T, D), dtype=np.float32)
    for c in range(N_CORES):
        b, h = c // 2, c % 2
        out[b, h * TC:(h + 1) * TC, :] = res.results[c]["out"]
    return out


# revision 12
# speedup vs baseline: 1.0631x; 1.0221x over previous
"""DepthAttentionResidual Trainium2 kernel.

Computation (see reference):
    ms      = mean(history^2, axis=-1)                      # [S,B,T]
    logits  = dot(query*rms_weight, history) * rsqrt(ms+eps)
    w       = softmax(logits, axis=S)
    out     = sum_s w[s] * history[s]                        # [B,T,D]

Sharding: data-parallel over (B=4) x (T halves) = 8 cores. Each core gets
hist [S=16, Tc=1024, D=1024] (64 MiB) and produces out [1024, 1024].

The per-core DMA subsystem measures ~235 GB/s regardless of descriptor
size (16 engines x ~14 B/ns, latency-bound), so the kernel is pinned at
~300 us of DMA wall time for its 68.4 MiB; everything else hides under
that. Engine budget per supertile (~21-23 us each, under the ~24 us DMA
period): ScalarE 16 Square+accum passes, VectorE 16 affine_mul_reduce
dot passes + softmax smalls, PE the masked-weight depth mix, GpSimd the
w2 build.

Per-core layout: partition p = s*8 + t' (S=16 depths x 8 t-blocks), D on
the free axis. A supertile is 128 t; slice g holds t_local = t'*16 + g.
  - rstd = rsqrt(ms+eps) via the int32 magic-constant seed + 2 Newton
    steps on VectorE: no Sqrt activation, so ScalarE only ever uses the
    exp_and_others table (Square/Exp/Copy) -> one ACT_TABLE_LOAD total
    (the baseline's Sqrt cost 2 x 1.28 us table swaps per supertile)
  - softmax over S: sum over s-partition-subgroups via 0/1-mask fp32
    matmuls (exact); reciprocal on VectorE
  - depth mix: per D-half, 16 accumulating fp32r matmuls (1 cycle/row)
    with block-expanded masked weights built on GpSimd (two-scalar
    tensor_scalar: e and 1/sumexp); PSUM drains split Scalar/Vector
    (GpSimd cannot access PSUM; SWDGE descriptor gen on the Q7s is also
    ~10x the spec cost, so all DMA stays on the SP/Act HWDGE rings)
  - query*rms_weight broadcast to 128 partitions on-chip via a
    1-partition ones matmul instead of a 256-packet broadcast DMA
Input DMAs alternate SP/Act HWDGE rings; output + masks ride Act.
The last supertile is split 64/64 to shorten the serial tail. fp32r
rounds operands to ~13 mantissa bits -> ~2e-4 relative output error.
"""
import numpy as np

import concourse.bass as bass
import concourse.bacc as bacc
import concourse.tile as tile
from concourse import mybir
from concourse import bass_utils

N_CORES = 8
S = 16
B = 4
T = 2048
D = 1024
EPS = 1e-5

TC = T // 2          # t positions per core
TG = 8               # t-blocks per partition set (S * TG = 128 partitions)
GROUPS = 16          # stat slices per supertile (one t per partition each)
J = 1                # consecutive t rows per DMA descriptor (J*4KiB each)
TS = TG * GROUPS     # t per supertile = 128
N_SUPER = TC // TS   # supertiles per core = 8
F32 = mybir.dt.float32
I32 = mybir.dt.int32
F32R = mybir.dt.float32r
RSQRT_MAGIC = 0x5F3759DF


def _build_program():
    nc = bacc.Bacc("TRN2", target_bir_lowering=False, debug=False,
                   enable_asserts=True, num_devices=N_CORES)

    hist = nc.dram_tensor("hist", [S, TC, D], F32R, kind="ExternalInput").ap()
    query = nc.dram_tensor("query", [D], F32, kind="ExternalInput").ap()
    rmsw = nc.dram_tensor("rms_weight", [D], F32, kind="ExternalInput").ap()
    mask_d = nc.dram_tensor("mask", [128, TG], F32, kind="ExternalInput").ap()
    maskt_d = nc.dram_tensor("maskT", [TG, 128], F32, kind="ExternalInput").ap()
    maskf_d = nc.dram_tensor("maskF", [128, GROUPS, 128], F32,
                             kind="ExternalInput").ap()
    maskf8_d = nc.dram_tensor("maskF8", [128, GROUPS // 2, 128 // 2], F32,
                              kind="ExternalInput").ap()
    out = nc.dram_tensor("out", [TC, D], F32, kind="ExternalOutput").ap()

    with tile.TileContext(nc) as tc:
        with (
            tc.tile_pool(name="singles", bufs=1) as singles,
            tc.tile_pool(name="hsup", bufs=2) as hpool,
            tc.tile_pool(name="stats", bufs=2) as stats,
            tc.tile_pool(name="w2", bufs=3) as w2pool,
            tc.tile_pool(name="outp", bufs=2) as outpool,
            tc.tile_pool(name="ps_stats", bufs=2, space="PSUM") as ps_stats,
            tc.tile_pool(name="ps_mix", bufs=2, space="PSUM") as ps_mix,
        ):
            # ---- constants ------------------------------------------------
            qw = singles.tile([128, D], F32)
            q1 = singles.tile([1, D], F32)
            w1 = singles.tile([1, D], F32)
            ones = singles.tile([1, 128], F32)
            mask = singles.tile([128, TG], F32)
            maskT = singles.tile([TG, 128], F32)
            maskF = singles.tile([128, GROUPS, 128], F32)
            maskF8 = singles.tile([128, GROUPS // 2, 128 // 2], F32)
            dummy_a = singles.tile([128, 1], F32)
            dummy_v = singles.tile([128, 1], F32)

            def emit_init():
                # constants ride the Act HWDGE ring behind nothing; the
                # input stream owns SP from t=0
                nc.scalar.dma_start(
                    out=q1[:],
                    in_=bass.AP(tensor=query.tensor, offset=0,
                                ap=[[0, 1], [1, D]]),
                )
                nc.scalar.dma_start(
                    out=w1[:],
                    in_=bass.AP(tensor=rmsw.tensor, offset=0,
                                ap=[[0, 1], [1, D]]),
                )
                nc.scalar.dma_start(out=mask[:], in_=mask_d)
                nc.scalar.dma_start(out=maskT[:], in_=maskt_d)
                nc.scalar.dma_start(out=maskF[:], in_=maskf_d)
                nc.scalar.dma_start(out=maskF8[:], in_=maskf8_d)
                nc.vector.memset(ones[:], 1.0)
                nc.vector.tensor_mul(q1[:], q1[:], w1[:])  # query * rms_weight
                # broadcast q1 to all 128 partitions: ones^T @ q1 via PE
                for c in range(2):
                    bc = ps_mix.tile([128, 512], F32, tag="m", name=f"bc{c}")
                    nc.tensor.matmul(out=bc[:], lhsT=ones[:],
                                     rhs=q1[:, c * 512:(c + 1) * 512],
                                     start=True, stop=True)
                    nc.scalar.activation(
                        out=qw[:, c * 512:(c + 1) * 512], in_=bc[:],
                        func=mybir.ActivationFunctionType.Copy,
                    )

            # ---- main loop over supertiles --------------------------------
            # last 128-t supertile is split in two 64-t halves to shorten
            # the serial tail after the final DMA
            emit_init()
            schedule = [(k * TS, GROUPS) for k in range(N_SUPER - 1)]
            schedule += [((N_SUPER - 1) * TS, GROUPS // 2),
                         ((N_SUPER - 1) * TS + TS // 2, GROUPS // 2)]
            dma_i = 0
            # Software pipelining: the in-order HWDGE queues head-block, so
            # an output DMA emitted right after supertile k's mix would
            # stall every input DMA queued behind it until the whole k
            # pipeline drains (measured: DMA idle 35% -> 458 us). Defer
            # PSUM drains by one iteration and output DMAs by two: their
            # waits are long-satisfied when the queue head reaches them.
            pending_drain = None   # (m_ps pair, t0, ts_k) from iter k-1
            pending_out = []       # [(t0, ts_k, ot tile), ...] FIFO

            def emit_drain(drain):
                m_ps_d, t0_d, ts_d = drain
                ot = outpool.tile([ts_d, D], F32, tag="ot", bufs=3)
                nc.scalar.activation(
                    out=ot[:, 0:512], in_=m_ps_d[0][:],
                    func=mybir.ActivationFunctionType.Copy,
                )
                nc.vector.tensor_copy(out=ot[:, 512:1024], in_=m_ps_d[1][:])
                pending_out.append((t0_d, ts_d, ot))

            def emit_out():
                nonlocal dma_i
                t0_o, ts_o, ot = pending_out.pop(0)
                eng = nc.sync if dma_i % 2 == 0 else nc.scalar
                dma_i += 1
                eng.dma_start(out=out[t0_o:t0_o + ts_o, :], in_=ot[:])

            for k, (t0, groups) in enumerate(schedule):
                ts_k = TG * groups
                ndma = groups // J
                half = ndma // 2

                # load [S, ts_k, D] as partitions (s, t') x free (g, d)
                # where t_local = t' * groups + g; one descriptor per
                # partition per dma_start, J*4KiB contiguous on both sides
                hsupA = hpool.tile([128, half, J, D], F32R, tag="hsupA",
                                   name="hsupA", bufs=3)
                hsupB = hpool.tile([128, ndma - half, J, D], F32R,
                                   tag="hsupB", name="hsupB")

                def hslice(g):
                    gd, j = g // J, g % J
                    tile_ = hsupA if gd < half else hsupB
                    return tile_[:, gd - half if gd >= half else gd, j, :]

                srcv = hist[:, t0:t0 + ts_k, :].rearrange(
                    "s (t gd j) d -> s t gd (j d)", t=TG, gd=ndma)
                for gd in range(ndma):
                    tile_ = hsupA if gd < half else hsupB
                    eng = nc.sync if dma_i % 2 == 0 else nc.scalar
                    dma_i += 1
                    eng.dma_start(
                        out=tile_[:, gd - half if gd >= half else gd, :, :]
                        .rearrange("p j d -> p (j d)"),
                        in_=srcv[:, :, gd, :])
                # output DMA for a finished supertile rides behind these
                # inputs; its wait is already satisfied at queue-head time
                if pending_out:
                    emit_out()

                # ScalarE also issues half the input DMAs (~5 us/supertile
                # of SEQ time), so VectorE picks up 2 of the 16 sumsq
                ss = stats.tile([128, groups], F32, tag="ss")
                dot = stats.tile([128, groups], F32, tag="dot")
                n_vec_sq = 2 if groups == GROUPS else 1
                for g in range(groups):
                    h_g = hslice(g).bitcast(F32)
                    if g < groups - n_vec_sq:
                        nc.scalar.activation(
                            out=dummy_a.broadcast_to([128, D]),
                            in_=h_g,
                            func=mybir.ActivationFunctionType.Square,
                            accum_out=ss[:, g:g + 1],
                        )
                    else:
                        nc.vector.affine_mul_reduce(
                            out=dummy_v.broadcast_to([128, D]),
                            accum_out=ss[:, g:g + 1],
                            in0=h_g, in1=h_g, scale=1.0, bias=0.0,
                        )
                    nc.vector.affine_mul_reduce(
                        out=dummy_v.broadcast_to([128, D]),
                        accum_out=dot[:, g:g + 1],
                        in0=h_g,
                        in1=qw[:],
                        scale=1.0,
                        bias=0.0,
                    )

                # drain supertile k-1's PSUM now: the mix that produced it
                # finished during our stats, so these copies don't stall
                if pending_drain is not None:
                    emit_drain(pending_drain)
                    pending_drain = None

                # rstd = rsqrt(ss/D + eps) via magic-constant + 2 Newton
                # steps, entirely on VectorE (no activation table needed)
                v = stats.tile([128, groups], F32, tag="v")
                nc.vector.tensor_scalar(
                    out=v[:], in0=ss[:], scalar1=1.0 / D, scalar2=EPS,
                    op0=mybir.AluOpType.mult, op1=mybir.AluOpType.add)
                y = stats.tile([128, groups], F32, tag="y")
                nc.vector.tensor_scalar(
                    out=y[:].bitcast(I32), in0=v[:].bitcast(I32),
                    scalar1=1, scalar2=None,
                    op0=mybir.AluOpType.logical_shift_right)
                nc.vector.tensor_scalar(
                    out=y[:].bitcast(I32), in0=y[:].bitcast(I32),
                    scalar1=-1, scalar2=RSQRT_MAGIC,
                    op0=mybir.AluOpType.mult, op1=mybir.AluOpType.add)
                t1 = stats.tile([128, groups], F32, tag="t1")
                t2 = stats.tile([128, groups], F32, tag="t2")
                for _ in range(2):  # y *= 1.5 - 0.5*v*y^2
                    nc.vector.tensor_mul(t1[:], y[:], y[:])
                    nc.vector.tensor_mul(t2[:], v[:], t1[:])
                    nc.vector.tensor_scalar(
                        out=t2[:], in0=t2[:], scalar1=-0.5, scalar2=1.5,
                        op0=mybir.AluOpType.mult, op1=mybir.AluOpType.add)
                    nc.vector.tensor_mul(y[:], y[:], t2[:])

                logit = stats.tile([128, groups], F32, tag="logit")
                nc.vector.tensor_mul(logit[:], dot[:], y[:])
                e = stats.tile([128, groups], F32, tag="e")
                nc.scalar.activation(
                    out=e[:], in_=logit[:],
                    func=mybir.ActivationFunctionType.Exp,
                )

                # sumexp over s: [8t', G] = mask^T @ e (exact fp32 matmul)
                se_ps = ps_stats.tile([TG, groups], F32, tag="se")
                nc.tensor.matmul(out=se_ps[:], lhsT=mask[:], rhs=e[:],
                                 start=True, stop=True)
                rse = stats.tile([TG, groups], F32, tag="rse")
                nc.vector.reciprocal(out=rse[:], in_=se_ps[:])
                # broadcast rse back to (s,t') partitions: maskT^T @ rse
                rseb_ps = ps_stats.tile([128, groups], F32, tag="rseb")
                nc.tensor.matmul(out=rseb_ps[:], lhsT=maskT[:], rhs=rse[:],
                                 start=True, stop=True)
                rseb = stats.tile([128, groups], F32, tag="rsebs")
                nc.vector.tensor_copy(out=rseb[:], in_=rseb_ps[:])

                # depth mix: accumulate 16 masked-weight matmuls per D chunk
                m_ps = [ps_mix.tile([TG * groups, 512], F32, tag="m", name=f"m{c}")
                        for c in range(2)]
                for g in range(groups):
                    w2 = w2pool.tile([128, TG * groups], F32R, tag="w2")
                    nc.gpsimd.tensor_scalar(
                        out=w2[:],
                        in0=(maskF[:, g, :] if groups == GROUPS
                             else maskF8[:, g, :]),
                        scalar1=e[:, g:g + 1],
                        scalar2=rseb[:, g:g + 1],
                        op0=mybir.AluOpType.mult,
                        op1=mybir.AluOpType.mult,
                    )
                    for c in range(2):
                        nc.tensor.matmul(
                            out=m_ps[c][:],
                            lhsT=w2[:],
                            rhs=hslice(g)[:, c * 512:(c + 1) * 512],
                            start=(g == 0),
                            stop=(g == groups - 1),
                        )

                pending_drain = (m_ps, t0, ts_k)

            # tail flush
            emit_drain(pending_drain)
            while pending_out:
                emit_out()

    nc.compile()
    return nc


_NC = None


def _get_program():
    global _NC
    if _NC is None:
        _NC = _build_program()
    return _NC


def _make_masks():
    # partition p = s*TG + t'; group slice g holds t_local = t'*GROUPS + g
    p = np.arange(128)
    mask = (p[:, None] % TG == np.arange(TG)[None, :]).astype(np.float32)
    maskF = np.zeros((128, GROUPS, 128), np.float32)
    for g in range(GROUPS):
        maskF[p, g, (p % TG) * GROUPS + g] = 1.0
    maskF8 = np.zeros((128, GROUPS // 2, 64), np.float32)
    for g in range(GROUPS // 2):
        maskF8[p, g, (p % TG) * (GROUPS // 2) + g] = 1.0
    return mask, np.ascontiguousarray(mask.T), maskF, maskF8


def kernel(history, query, rms_weight):
    history = np.asarray(history, dtype=np.float32)
    query = np.asarray(query, dtype=np.float32)
    rms_weight = np.asarray(rms_weight, dtype=np.float32)
    assert history.shape == (S, B, T, D), history.shape

    nc = _get_program()
    mask, maskT, maskF, maskF8 = _make_masks()

    in_maps = []
    for c in range(N_CORES):
        b, h = c // 2, c % 2
        shard = np.ascontiguousarray(history[:, b, h * TC:(h + 1) * TC, :])
        in_maps.append({
            "hist": shard,
            "query": query,
            "rms_weight": rms_weight,
            "mask": mask,
            "maskT": maskT,
            "maskF": maskF,
            "maskF8": maskF8,
        })

    res = bass_utils.run_bass_kernel_spmd(nc, in_maps, list(range(N_CORES)))

    out = np.empty((B, T, D), dtype=np.float32)
    for c in range(N_CORES):
        b, h = c // 2, c % 2
        out[b, h * TC:(h + 1) * TC, :] = res.results[c]["out"]
    return out


# revision 14
# speedup vs baseline: 1.1042x; 1.0386x over previous
"""DepthAttentionResidual Trainium2 kernel.

Computation (see reference):
    ms      = mean(history^2, axis=-1)                      # [S,B,T]
    logits  = dot(query*rms_weight, history) * rsqrt(ms+eps)
    w       = softmax(logits, axis=S)
    out     = sum_s w[s] * history[s]                        # [B,T,D]

Sharding: data-parallel over (B=4) x (T halves) = 8 cores. Each core gets
hist [S=16, Tc=1024, D=1024] (64 MiB) and produces out [1024, 1024].

The per-core DMA subsystem measures ~235 GB/s regardless of descriptor
size (16 engines x ~14 B/ns, latency-bound), so the kernel is pinned at
~300 us of DMA wall time for its 68.4 MiB; everything else hides under
that. Engine budget per supertile (~21-23 us each, under the ~24 us DMA
period): ScalarE 16 Square+accum passes, VectorE 16 affine_mul_reduce
dot passes + softmax smalls, PE the masked-weight depth mix, GpSimd the
w2 build.

Per-core layout: partition p = s*8 + t' (S=16 depths x 8 t-blocks), D on
the free axis. A supertile is 128 t; slice g holds t_local = t'*16 + g.
  - rstd = rsqrt(ms+eps) via the int32 magic-constant seed + 2 Newton
    steps on VectorE: no Sqrt activation, so ScalarE only ever uses the
    exp_and_others table (Square/Exp/Copy) -> one ACT_TABLE_LOAD total
    (the baseline's Sqrt cost 2 x 1.28 us table swaps per supertile)
  - softmax over S: sum over s-partition-subgroups via 0/1-mask fp32
    matmuls (exact); reciprocal on VectorE
  - depth mix: per D-half, 16 accumulating fp32r matmuls (1 cycle/row)
    with block-expanded masked weights built on GpSimd (two-scalar
    tensor_scalar: e and 1/sumexp); PSUM drains split Scalar/Vector
    (GpSimd cannot access PSUM; SWDGE descriptor gen on the Q7s is also
    ~10x the spec cost, so all DMA stays on the SP/Act HWDGE rings)
  - query*rms_weight broadcast to 128 partitions on-chip via a
    1-partition ones matmul instead of a 256-packet broadcast DMA
Input DMAs alternate SP/Act HWDGE rings; output + masks ride Act.
The last supertile is split 64/64 to shorten the serial tail. fp32r
rounds operands to ~13 mantissa bits -> ~2e-4 relative output error.
"""
import numpy as np

import concourse.bass as bass
import concourse.bacc as bacc
import concourse.tile as tile
from concourse import mybir
from concourse import bass_utils

N_CORES = 8
S = 16
B = 4
T = 2048
D = 1024
EPS = 1e-5

TC = T // 2          # t positions per core
TG = 8               # t-blocks per partition set (S * TG = 128 partitions)
GROUPS = 16          # stat slices per supertile (one t per partition each)
J = 1                # consecutive t rows per DMA descriptor (J*4KiB each)
TS = TG * GROUPS     # t per supertile = 128
N_SUPER = TC // TS   # supertiles per core = 8
F32 = mybir.dt.float32
I32 = mybir.dt.int32
F32R = mybir.dt.float32r
RSQRT_MAGIC = 0x5F3759DF


def _build_program():
    nc = bacc.Bacc("TRN2", target_bir_lowering=False, debug=False,
                   enable_asserts=True, num_devices=N_CORES)

    hist = nc.dram_tensor("hist", [S, TC, D], F32R, kind="ExternalInput").ap()
    query = nc.dram_tensor("query", [D], F32, kind="ExternalInput").ap()
    rmsw = nc.dram_tensor("rms_weight", [D], F32, kind="ExternalInput").ap()
    mask_d = nc.dram_tensor("mask", [128, TG], F32, kind="ExternalInput").ap()
    maskt_d = nc.dram_tensor("maskT", [TG, 128], F32, kind="ExternalInput").ap()
    maskf_d = nc.dram_tensor("maskF", [128, GROUPS, 128], F32,
                             kind="ExternalInput").ap()
    maskf8_d = nc.dram_tensor("maskF8", [128, GROUPS // 2, 128 // 2], F32,
                              kind="ExternalInput").ap()
    out = nc.dram_tensor("out", [TC, D], F32, kind="ExternalOutput").ap()

    with tile.TileContext(nc) as tc:
        with (
            tc.tile_pool(name="singles", bufs=1) as singles,
            tc.tile_pool(name="hsup", bufs=2) as hpool,
            tc.tile_pool(name="stats", bufs=2) as stats,
            tc.tile_pool(name="w2", bufs=3) as w2pool,
            tc.tile_pool(name="outp", bufs=2) as outpool,
            tc.tile_pool(name="ps_stats", bufs=2, space="PSUM") as ps_stats,
            tc.tile_pool(name="ps_mix", bufs=2, space="PSUM") as ps_mix,
        ):
            # ---- constants ------------------------------------------------
            qw = singles.tile([128, D], F32)
            q1 = singles.tile([1, D], F32)
            w1 = singles.tile([1, D], F32)
            ones = singles.tile([1, 128], F32)
            mask = singles.tile([128, TG], F32)
            maskT = singles.tile([TG, 128], F32)
            maskF = singles.tile([128, GROUPS, 128], F32)
            maskF8 = singles.tile([128, GROUPS // 2, 128 // 2], F32)
            dummy_a = singles.tile([128, 1], F32)
            dummy_v = singles.tile([128, 1], F32)

            def emit_init():
                # constants ride the Act HWDGE ring behind nothing; the
                # input stream owns SP from t=0
                nc.scalar.dma_start(
                    out=q1[:],
                    in_=bass.AP(tensor=query.tensor, offset=0,
                                ap=[[0, 1], [1, D]]),
                )
                nc.scalar.dma_start(
                    out=w1[:],
                    in_=bass.AP(tensor=rmsw.tensor, offset=0,
                                ap=[[0, 1], [1, D]]),
                )
                nc.scalar.dma_start(out=mask[:], in_=mask_d)
                nc.scalar.dma_start(out=maskT[:], in_=maskt_d)
                nc.scalar.dma_start(out=maskF[:], in_=maskf_d)
                nc.scalar.dma_start(out=maskF8[:], in_=maskf8_d)
                nc.vector.memset(ones[:], 1.0)
                nc.vector.tensor_mul(q1[:], q1[:], w1[:])  # query * rms_weight
                # broadcast q1 to all 128 partitions: ones^T @ q1 via PE
                for c in range(2):
                    bc = ps_mix.tile([128, 512], F32, tag="m", name=f"bc{c}")
                    nc.tensor.matmul(out=bc[:], lhsT=ones[:],
                                     rhs=q1[:, c * 512:(c + 1) * 512],
                                     start=True, stop=True)
                    nc.scalar.activation(
                        out=qw[:, c * 512:(c + 1) * 512], in_=bc[:],
                        func=mybir.ActivationFunctionType.Copy,
                    )

            # ---- main loop over supertiles --------------------------------
            # last 128-t supertile is split in two 64-t halves to shorten
            # the serial tail after the final DMA
            emit_init()
            schedule = [(k * TS, GROUPS) for k in range(N_SUPER - 1)]
            schedule += [((N_SUPER - 1) * TS, GROUPS // 2),
                         ((N_SUPER - 1) * TS + TS // 2, GROUPS // 2)]
            dma_i = 0
            # Software pipelining: the in-order HWDGE queues head-block, so
            # an output DMA emitted right after supertile k's mix would
            # stall every input DMA queued behind it until the whole k
            # pipeline drains (measured: DMA idle 35% -> 458 us). Defer
            # PSUM drains by one iteration and output DMAs by two: their
            # waits are long-satisfied when the queue head reaches them.
            pending_drain = None   # (m_ps pair, t0, ts_k) from iter k-1
            pending_out = []       # [(t0, ts_k, ot tile), ...] FIFO

            def emit_drain(drain):
                m_ps_d, t0_d, ts_d = drain
                ot = outpool.tile([ts_d, D], F32, tag="ot", bufs=2)
                nc.scalar.activation(
                    out=ot[:, 0:512], in_=m_ps_d[0][:],
                    func=mybir.ActivationFunctionType.Copy,
                )
                nc.vector.tensor_copy(out=ot[:, 512:1024], in_=m_ps_d[1][:])
                pending_out.append((t0_d, ts_d, ot))

            def emit_out():
                nonlocal dma_i
                t0_o, ts_o, ot = pending_out.pop(0)
                eng = nc.sync if dma_i % 2 == 0 else nc.scalar
                dma_i += 1
                eng.dma_start(out=out[t0_o:t0_o + ts_o, :], in_=ot[:])

            for k, (t0, groups) in enumerate(schedule):
                ts_k = TG * groups
                ndma = groups // J
                half = ndma // 2

                # load [S, ts_k, D] as partitions (s, t') x free (g, d)
                # where t_local = t' * groups + g; one descriptor per
                # partition per dma_start, J*4KiB contiguous on both sides
                hsupA = hpool.tile([128, half, J, D], F32R, tag="hsupA",
                                   name="hsupA", bufs=3)
                hsupB = hpool.tile([128, ndma - half, J, D], F32R,
                                   tag="hsupB", name="hsupB")

                def hslice(g):
                    gd, j = g // J, g % J
                    tile_ = hsupA if gd < half else hsupB
                    return tile_[:, gd - half if gd >= half else gd, j, :]

                srcv = hist[:, t0:t0 + ts_k, :].rearrange(
                    "s (t gd j) d -> s t gd (j d)", t=TG, gd=ndma)
                for gd in range(ndma):
                    tile_ = hsupA if gd < half else hsupB
                    eng = nc.sync if dma_i % 2 == 0 else nc.scalar
                    dma_i += 1
                    eng.dma_start(
                        out=tile_[:, gd - half if gd >= half else gd, :, :]
                        .rearrange("p j d -> p (j d)"),
                        in_=srcv[:, :, gd, :])
                # output DMA for a finished supertile rides behind these
                # inputs; its wait is already satisfied at queue-head time
                if pending_out:
                    emit_out()

                # Two 8-group waves per supertile: wave 0's softmax + mix
                # run on PE/GpSimd while wave 1's stats still stream on
                # Scalar/Vector, so the mix finishes ~10 us earlier and
                # frees the h buffers the next-next input DMAs wait on.
                ss = stats.tile([128, groups], F32, tag="ss")
                dot = stats.tile([128, groups], F32, tag="dot")
                m_ps = [ps_mix.tile([TG * groups, 512], F32, tag="m", name=f"m{c}")
                        for c in range(2)]
                n_vec_sq = 2 if groups == GROUPS else 1
                WAVE = 8
                for w0 in range(0, groups, WAVE):
                    w1 = min(w0 + WAVE, groups)
                    gw = w1 - w0
                    for g in range(w0, w1):
                        h_g = hslice(g).bitcast(F32)
                        if g < groups - n_vec_sq:
                            nc.scalar.activation(
                                out=dummy_a.broadcast_to([128, D]),
                                in_=h_g,
                                func=mybir.ActivationFunctionType.Square,
                                accum_out=ss[:, g:g + 1],
                            )
                        else:
                            nc.vector.affine_mul_reduce(
                                out=dummy_v.broadcast_to([128, D]),
                                accum_out=ss[:, g:g + 1],
                                in0=h_g, in1=h_g, scale=1.0, bias=0.0,
                            )
                        nc.vector.affine_mul_reduce(
                            out=dummy_v.broadcast_to([128, D]),
                            accum_out=dot[:, g:g + 1],
                            in0=h_g,
                            in1=qw[:],
                            scale=1.0,
                            bias=0.0,
                        )

                    # drain supertile k-1's PSUM behind wave 0's stats: the
                    # mix that produced it finished during these stats
                    if pending_drain is not None:
                        emit_drain(pending_drain)
                        pending_drain = None

                    # rstd = rsqrt(ss/D + eps) via magic-constant + 2
                    # Newton steps on VectorE (no activation table needed)
                    sw = slice(w0, w1)
                    v = stats.tile([128, gw], F32, tag="v")
                    nc.vector.tensor_scalar(
                        out=v[:], in0=ss[:, sw], scalar1=1.0 / D, scalar2=EPS,
                        op0=mybir.AluOpType.mult, op1=mybir.AluOpType.add)
                    y = stats.tile([128, gw], F32, tag="y")
                    nc.vector.tensor_scalar(
                        out=y[:].bitcast(I32), in0=v[:].bitcast(I32),
                        scalar1=1, scalar2=None,
                        op0=mybir.AluOpType.logical_shift_right)
                    nc.vector.tensor_scalar(
                        out=y[:].bitcast(I32), in0=y[:].bitcast(I32),
                        scalar1=-1, scalar2=RSQRT_MAGIC,
                        op0=mybir.AluOpType.mult, op1=mybir.AluOpType.add)
                    t1 = stats.tile([128, gw], F32, tag="t1")
                    t2 = stats.tile([128, gw], F32, tag="t2")
                    for _ in range(2):  # y *= 1.5 - 0.5*v*y^2
                        nc.vector.tensor_mul(t1[:], y[:], y[:])
                        nc.vector.tensor_mul(t2[:], v[:], t1[:])
                        nc.vector.tensor_scalar(
                            out=t2[:], in0=t2[:], scalar1=-0.5, scalar2=1.5,
                            op0=mybir.AluOpType.mult, op1=mybir.AluOpType.add)
                        nc.vector.tensor_mul(y[:], y[:], t2[:])

                    logit = stats.tile([128, gw], F32, tag="logit")
                    nc.vector.tensor_mul(logit[:], dot[:, sw], y[:])
                    e = stats.tile([128, gw], F32, tag="e")
                    nc.scalar.activation(
                        out=e[:], in_=logit[:],
                        func=mybir.ActivationFunctionType.Exp,
                    )

                    # sumexp over s: [8t', gw] = mask^T @ e (exact fp32)
                    se_ps = ps_stats.tile([TG, gw], F32, tag="se")
                    nc.tensor.matmul(out=se_ps[:], lhsT=mask[:], rhs=e[:],
                                     start=True, stop=True)
                    rse = stats.tile([TG, gw], F32, tag="rse")
                    nc.vector.reciprocal(out=rse[:], in_=se_ps[:])
                    # broadcast rse back to (s,t'): maskT^T @ rse
                    rseb_ps = ps_stats.tile([128, gw], F32, tag="rseb")
                    nc.tensor.matmul(out=rseb_ps[:], lhsT=maskT[:], rhs=rse[:],
                                     start=True, stop=True)
                    rseb = stats.tile([128, gw], F32, tag="rsebs")
                    nc.vector.tensor_copy(out=rseb[:], in_=rseb_ps[:])

                    # depth mix: accumulating fp32r matmuls per D chunk
                    for gi in range(gw):
                        g = w0 + gi
                        w2 = w2pool.tile([128, TG * groups], F32R, tag="w2")
                        nc.gpsimd.tensor_scalar(
                            out=w2[:],
                            in0=(maskF[:, g, :] if groups == GROUPS
                                 else maskF8[:, g, :]),
                            scalar1=e[:, gi:gi + 1],
                            scalar2=rseb[:, gi:gi + 1],
                            op0=mybir.AluOpType.mult,
                            op1=mybir.AluOpType.mult,
                        )
                        for c in range(2):
                            nc.tensor.matmul(
                                out=m_ps[c][:],
                                lhsT=w2[:],
                                rhs=hslice(g)[:, c * 512:(c + 1) * 512],
                                start=(g == 0),
                                stop=(g == groups - 1),
                            )

                pending_drain = (m_ps, t0, ts_k)

            # tail flush
            emit_drain(pending_drain)
            while pending_out:
                emit_out()

    nc.compile()
    return nc


_NC = None


def _get_program():
    global _NC
    if _NC is None:
        _NC = _build_program()
    return _NC


def _make_masks():
    # partition p = s*TG + t'; group slice g holds t_local = t'*GROUPS + g
    p = np.arange(128)
    mask = (p[:, None] % TG == np.arange(TG)[None, :]).astype(np.float32)
    maskF = np.zeros((128, GROUPS, 128), np.float32)
    for g in range(GROUPS):
        maskF[p, g, (p % TG) * GROUPS + g] = 1.0
    maskF8 = np.zeros((128, GROUPS // 2, 64), np.float32)
    for g in range(GROUPS // 2):
        maskF8[p, g, (p % TG) * (GROUPS // 2) + g] = 1.0
    return mask, np.ascontiguousarray(mask.T), maskF, maskF8


def kernel(history, query, rms_weight):
    history = np.asarray(history, dtype=np.float32)
    query = np.asarray(query, dtype=np.float32)
    rms_weight = np.asarray(rms_weight, dtype=np.float32)
    assert history.shape == (S, B, T, D), history.shape

    nc = _get_program()
    mask, maskT, maskF, maskF8 = _make_masks()

    in_maps = []
    for c in range(N_CORES):
        b, h = c // 2, c % 2
        shard = np.ascontiguousarray(history[:, b, h * TC:(h + 1) * TC, :])
        in_maps.append({
            "hist": shard,
            "query": query,
            "rms_weight": rms_weight,
            "mask": mask,
            "maskT": maskT,
            "maskF": maskF,
            "maskF8": maskF8,
        })

    res = bass_utils.run_bass_kernel_spmd(nc, in_maps, list(range(N_CORES)))

    out = np.empty((B, T, D), dtype=np.float32)
    for c in range(N_CORES):
        b, h = c // 2, c % 2
        out[b, h * TC:(h + 1) * TC, :] = res.results[c]["out"]
    return out


# revision 17
# speedup vs baseline: 1.1415x; 1.0338x over previous
"""DepthAttentionResidual Trainium2 kernel.

Computation (see reference):
    ms      = mean(history^2, axis=-1)                      # [S,B,T]
    logits  = dot(query*rms_weight, history) * rsqrt(ms+eps)
    w       = softmax(logits, axis=S)
    out     = sum_s w[s] * history[s]                        # [B,T,D]

Sharding: data-parallel over (B=4) x (T halves) = 8 cores. Each core gets
hist [S=16, Tc=1024, D=1024] (64 MiB) and produces out [1024, 1024].

The per-core DMA subsystem measures ~235 GB/s regardless of descriptor
size (16 engines x ~14 B/ns, latency-bound), so the kernel is pinned at
~300 us of DMA wall time for its 68.4 MiB; everything else hides under
that. Engine budget per supertile (~21-23 us each, under the ~24 us DMA
period): ScalarE 16 Square+accum passes, VectorE 16 affine_mul_reduce
dot passes + softmax smalls, PE the masked-weight depth mix, GpSimd the
w2 build.

Per-core layout: partition p = s*8 + t' (S=16 depths x 8 t-blocks), D on
the free axis. A supertile is 128 t; slice g holds t_local = t'*16 + g.
  - rstd = rsqrt(ms+eps) via the int32 magic-constant seed + 2 Newton
    steps on VectorE: no Sqrt activation, so ScalarE only ever uses the
    exp_and_others table (Square/Exp/Copy) -> one ACT_TABLE_LOAD total
    (the baseline's Sqrt cost 2 x 1.28 us table swaps per supertile)
  - softmax over S: sum over s-partition-subgroups via 0/1-mask fp32
    matmuls (exact); reciprocal on VectorE
  - depth mix: per D-half, 16 accumulating fp32r matmuls (1 cycle/row)
    with block-expanded masked weights built on GpSimd (two-scalar
    tensor_scalar: e and 1/sumexp); PSUM drains split Scalar/Vector
    (GpSimd cannot access PSUM; SWDGE descriptor gen on the Q7s is also
    ~10x the spec cost, so all DMA stays on the SP/Act HWDGE rings)
  - query*rms_weight broadcast to 128 partitions on-chip via a
    1-partition ones matmul instead of a 256-packet broadcast DMA
Input DMAs alternate SP/Act HWDGE rings; output + masks ride Act.
The last supertile is split 64/64 to shorten the serial tail. fp32r
rounds operands to ~13 mantissa bits -> ~2e-4 relative output error.
"""
import numpy as np

import concourse.bass as bass
import concourse.bacc as bacc
import concourse.tile as tile
from concourse import mybir
from concourse import bass_utils

N_CORES = 8
S = 16
B = 4
T = 2048
D = 1024
EPS = 1e-5

TC = T // 2          # t positions per core
TG = 8               # t-blocks per partition set (S * TG = 128 partitions)
GROUPS = 16          # stat slices per supertile (one t per partition each)
J = 1                # consecutive t rows per DMA descriptor (J*4KiB each)
TS = TG * GROUPS     # t per supertile = 128
N_SUPER = TC // TS   # supertiles per core = 8
F32 = mybir.dt.float32
I32 = mybir.dt.int32
F32R = mybir.dt.float32r
RSQRT_MAGIC = 0x5F3759DF


def _build_program():
    nc = bacc.Bacc("TRN2", target_bir_lowering=False, debug=False,
                   enable_asserts=True, num_devices=N_CORES)

    hist = nc.dram_tensor("hist", [S, TC, D], F32R, kind="ExternalInput").ap()
    query = nc.dram_tensor("query", [D], F32, kind="ExternalInput").ap()
    rmsw = nc.dram_tensor("rms_weight", [D], F32, kind="ExternalInput").ap()
    mask_d = nc.dram_tensor("mask", [128, TG], F32, kind="ExternalInput").ap()
    maskt_d = nc.dram_tensor("maskT", [TG, 128], F32, kind="ExternalInput").ap()
    maskf_d = nc.dram_tensor("maskF", [128, GROUPS, 128], F32,
                             kind="ExternalInput").ap()
    maskf8_d = nc.dram_tensor("maskF8", [128, GROUPS // 2, 128 // 2], F32,
                              kind="ExternalInput").ap()
    out = nc.dram_tensor("out", [TC, D], F32, kind="ExternalOutput").ap()

    with tile.TileContext(nc) as tc:
        with (
            tc.tile_pool(name="singles", bufs=1) as singles,
            tc.tile_pool(name="hsup", bufs=2) as hpool,
            tc.tile_pool(name="stats", bufs=2) as stats,
            tc.tile_pool(name="w2", bufs=3) as w2pool,
            tc.tile_pool(name="outp", bufs=2) as outpool,
            tc.tile_pool(name="ps_stats", bufs=2, space="PSUM") as ps_stats,
            tc.tile_pool(name="ps_mix", bufs=2, space="PSUM") as ps_mix,
        ):
            # ---- constants ------------------------------------------------
            qw = singles.tile([128, D], F32)
            q1 = singles.tile([1, D], F32)
            w1 = singles.tile([1, D], F32)
            ones = singles.tile([1, 128], F32)
            mask = singles.tile([128, TG], F32)
            maskT = singles.tile([TG, 128], F32)
            maskF = singles.tile([128, GROUPS, 128], F32)
            maskF8 = singles.tile([128, GROUPS // 2, 128 // 2], F32)
            dummy_a = singles.tile([128, 1], F32)
            dummy_v = singles.tile([128, 1], F32)

            def emit_init():
                # constants ride the Act HWDGE ring behind nothing; the
                # input stream owns SP from t=0
                nc.scalar.dma_start(
                    out=q1[:],
                    in_=bass.AP(tensor=query.tensor, offset=0,
                                ap=[[0, 1], [1, D]]),
                )
                nc.scalar.dma_start(
                    out=w1[:],
                    in_=bass.AP(tensor=rmsw.tensor, offset=0,
                                ap=[[0, 1], [1, D]]),
                )
                nc.scalar.dma_start(out=mask[:], in_=mask_d)
                nc.scalar.dma_start(out=maskT[:], in_=maskt_d)
                nc.scalar.dma_start(out=maskF[:], in_=maskf_d)
                nc.scalar.dma_start(out=maskF8[:], in_=maskf8_d)
                nc.vector.memset(ones[:], 1.0)
                nc.vector.tensor_mul(q1[:], q1[:], w1[:])  # query * rms_weight
                # broadcast q1 to all 128 partitions: ones^T @ q1 via PE
                for c in range(2):
                    bc = ps_mix.tile([128, 512], F32, tag="m", name=f"bc{c}")
                    nc.tensor.matmul(out=bc[:], lhsT=ones[:],
                                     rhs=q1[:, c * 512:(c + 1) * 512],
                                     start=True, stop=True)
                    nc.scalar.activation(
                        out=qw[:, c * 512:(c + 1) * 512], in_=bc[:],
                        func=mybir.ActivationFunctionType.Copy,
                    )

            # ---- main loop over supertiles --------------------------------
            # last 128-t supertile is split in two 64-t halves to shorten
            # the serial tail after the final DMA
            emit_init()
            schedule = [(k * TS, GROUPS) for k in range(N_SUPER - 1)]
            schedule += [((N_SUPER - 1) * TS, GROUPS // 2),
                         ((N_SUPER - 1) * TS + TS // 2, GROUPS // 2)]
            dma_i = 0
            # Software pipelining: the in-order HWDGE queues head-block, so
            # an output DMA emitted right after supertile k's mix would
            # stall every input DMA queued behind it until the whole k
            # pipeline drains (measured: DMA idle 35% -> 458 us). Defer
            # PSUM drains by one iteration and output DMAs by two: their
            # waits are long-satisfied when the queue head reaches them.
            pending_drain = None   # (m_ps pair, t0, ts_k) from iter k-1
            pending_out = []       # [(t0, ts_k, ot tile), ...] FIFO

            def emit_drain(drain):
                m_ps_d, t0_d, ts_d = drain
                ot = outpool.tile([ts_d, D], F32, tag="ot", bufs=3)
                nc.scalar.activation(
                    out=ot[:, 0:512], in_=m_ps_d[0][:],
                    func=mybir.ActivationFunctionType.Copy,
                )
                nc.vector.tensor_copy(out=ot[:, 512:1024], in_=m_ps_d[1][:])
                pending_out.append((t0_d, ts_d, ot))

            def emit_out():
                nonlocal dma_i
                t0_o, ts_o, ot = pending_out.pop(0)
                eng = nc.sync if dma_i % 2 == 0 else nc.scalar
                dma_i += 1
                eng.dma_start(out=out[t0_o:t0_o + ts_o, :], in_=ot[:])

            for k, (t0, groups) in enumerate(schedule):
                ts_k = TG * groups
                ndma = groups // J
                half = ndma // 2

                # load [S, ts_k, D] as partitions (s, t') x free (g, d)
                # where t_local = t' * groups + g; one 4KiB descriptor per
                # partition per dma_start (the measured DMA sweet spot).
                # h lives in QUARTER-supertile tiles (4 slices each) so the
                # mix frees buffers incrementally: input prefetch then
                # never waits for a whole supertile's mix to finish.
                nq = (ndma + 3) // 4
                hq = [hpool.tile([128, 4, D], F32R, tag=f"hq{q}",
                                 name=f"hq{q}", bufs=3 if q == 0 else 2)
                      for q in range(nq)]

                def hslice(g):
                    return hq[g // 4][:, g % 4, :]

                srcv = hist[:, t0:t0 + ts_k, :].rearrange(
                    "s (t gd j) d -> s t gd (j d)", t=TG, gd=ndma)
                for gd in range(ndma):
                    eng = nc.sync if dma_i % 2 == 0 else nc.scalar
                    dma_i += 1
                    eng.dma_start(
                        out=hq[gd // 4][:, gd % 4, :],
                        in_=srcv[:, :, gd, :])
                # output DMA for a finished supertile rides behind these
                # inputs. Hold TWO pending outputs: the queues prefetch
                # ~1.5 supertiles ahead, so a 2-iteration deferral still
                # races the drain and head-blocks the ring.
                if len(pending_out) > 1:
                    emit_out()

                # Two 8-group waves per supertile: wave 0's softmax + mix
                # run on PE/GpSimd while wave 1's stats still stream on
                # Scalar/Vector, so the mix finishes ~10 us earlier and
                # frees the h buffers the next-next input DMAs wait on.
                ss = stats.tile([128, groups], F32, tag="ss")
                dot = stats.tile([128, groups], F32, tag="dot")
                m_ps = [ps_mix.tile([TG * groups, 512], F32, tag="m", name=f"m{c}")
                        for c in range(2)]
                n_vec_sq = 2 if groups == GROUPS else 1
                WAVE = 8
                for w0 in range(0, groups, WAVE):
                    w1 = min(w0 + WAVE, groups)
                    gw = w1 - w0
                    for g in range(w0, w1):
                        h_g = hslice(g).bitcast(F32)
                        if g < groups - n_vec_sq:
                            nc.scalar.activation(
                                out=dummy_a.broadcast_to([128, D]),
                                in_=h_g,
                                func=mybir.ActivationFunctionType.Square,
                                accum_out=ss[:, g:g + 1],
                            )
                        else:
                            nc.vector.affine_mul_reduce(
                                out=dummy_v.broadcast_to([128, D]),
                                accum_out=ss[:, g:g + 1],
                                in0=h_g, in1=h_g, scale=1.0, bias=0.0,
                            )
                        nc.vector.affine_mul_reduce(
                            out=dummy_v.broadcast_to([128, D]),
                            accum_out=dot[:, g:g + 1],
                            in0=h_g,
                            in1=qw[:],
                            scale=1.0,
                            bias=0.0,
                        )

                    # drain supertile k-1's PSUM behind wave 0's stats: the
                    # mix that produced it finished during these stats
                    if pending_drain is not None:
                        emit_drain(pending_drain)
                        pending_drain = None

                    # rstd = rsqrt(ss/D + eps) via magic-constant + 2
                    # Newton steps on VectorE (no activation table needed)
                    sw = slice(w0, w1)
                    v = stats.tile([128, gw], F32, tag="v")
                    nc.vector.tensor_scalar(
                        out=v[:], in0=ss[:, sw], scalar1=1.0 / D, scalar2=EPS,
                        op0=mybir.AluOpType.mult, op1=mybir.AluOpType.add)
                    y = stats.tile([128, gw], F32, tag="y")
                    nc.vector.tensor_scalar(
                        out=y[:].bitcast(I32), in0=v[:].bitcast(I32),
                        scalar1=1, scalar2=None,
                        op0=mybir.AluOpType.logical_shift_right)
                    nc.vector.tensor_scalar(
                        out=y[:].bitcast(I32), in0=y[:].bitcast(I32),
                        scalar1=-1, scalar2=RSQRT_MAGIC,
                        op0=mybir.AluOpType.mult, op1=mybir.AluOpType.add)
                    t1 = stats.tile([128, gw], F32, tag="t1")
                    t2 = stats.tile([128, gw], F32, tag="t2")
                    for _ in range(2):  # y *= 1.5 - 0.5*v*y^2
                        nc.vector.tensor_mul(t1[:], y[:], y[:])
                        nc.vector.tensor_mul(t2[:], v[:], t1[:])
                        nc.vector.tensor_scalar(
                            out=t2[:], in0=t2[:], scalar1=-0.5, scalar2=1.5,
                            op0=mybir.AluOpType.mult, op1=mybir.AluOpType.add)
                        nc.vector.tensor_mul(y[:], y[:], t2[:])

                    logit = stats.tile([128, gw], F32, tag="logit")
                    nc.vector.tensor_mul(logit[:], dot[:, sw], y[:])
                    e = stats.tile([128, gw], F32, tag="e")
                    nc.scalar.activation(
                        out=e[:], in_=logit[:],
                        func=mybir.ActivationFunctionType.Exp,
                    )

                    # sumexp over s: [8t', gw] = mask^T @ e (exact fp32)
                    se_ps = ps_stats.tile([TG, gw], F32, tag="se")
                    nc.tensor.matmul(out=se_ps[:], lhsT=mask[:], rhs=e[:],
                                     start=True, stop=True)
                    rse = stats.tile([TG, gw], F32, tag="rse")
                    nc.vector.reciprocal(out=rse[:], in_=se_ps[:])
                    # broadcast rse back to (s,t'): maskT^T @ rse
                    rseb_ps = ps_stats.tile([128, gw], F32, tag="rseb")
                    nc.tensor.matmul(out=rseb_ps[:], lhsT=maskT[:], rhs=rse[:],
                                     start=True, stop=True)
                    rseb = stats.tile([128, gw], F32, tag="rsebs")
                    nc.vector.tensor_copy(out=rseb[:], in_=rseb_ps[:])

                    # depth mix: accumulating fp32r matmuls per D chunk
                    for gi in range(gw):
                        g = w0 + gi
                        w2 = w2pool.tile([128, TG * groups], F32R, tag="w2")
                        nc.gpsimd.tensor_scalar(
                            out=w2[:],
                            in0=(maskF[:, g, :] if groups == GROUPS
                                 else maskF8[:, g, :]),
                            scalar1=e[:, gi:gi + 1],
                            scalar2=rseb[:, gi:gi + 1],
                            op0=mybir.AluOpType.mult,
                            op1=mybir.AluOpType.mult,
                        )
                        for c in range(2):
                            nc.tensor.matmul(
                                out=m_ps[c][:],
                                lhsT=w2[:],
                                rhs=hslice(g)[:, c * 512:(c + 1) * 512],
                                start=(g == 0),
                                stop=(g == groups - 1),
                            )

                pending_drain = (m_ps, t0, ts_k)

            # tail flush
            emit_drain(pending_drain)
            while pending_out:
                emit_out()

    nc.compile()
    return nc


_NC = None


def _get_program():
    global _NC
    if _NC is None:
        _NC = _build_program()
    return _NC


def _make_masks():
    # partition p = s*TG + t'; group slice g holds t_local = t'*GROUPS + g
    p = np.arange(128)
    mask = (p[:, None] % TG == np.arange(TG)[None, :]).astype(np.float32)
    maskF = np.zeros((128, GROUPS, 128), np.float32)
    for g in range(GROUPS):
        maskF[p, g, (p % TG) * GROUPS + g] = 1.0
    maskF8 = np.zeros((128, GROUPS // 2, 64), np.float32)
    for g in range(GROUPS // 2):
        maskF8[p, g, (p % TG) * (GROUPS // 2) + g] = 1.0
    return mask, np.ascontiguousarray(mask.T), maskF, maskF8


def kernel(history, query, rms_weight):
    history = np.asarray(history, dtype=np.float32)
    query = np.asarray(query, dtype=np.float32)
    rms_weight = np.asarray(rms_weight, dtype=np.float32)
    assert history.shape == (S, B, T, D), history.shape

    nc = _get_program()
    mask, maskT, maskF, maskF8 = _make_masks()

    in_maps = []
    for c in range(N_CORES):
        b, h = c // 2, c % 2
        shard = np.ascontiguousarray(history[:, b, h * TC:(h + 1) * TC, :])
        in_maps.append({
            "hist": shard,
            "query": query,
            "rms_weight": rms_weight,
            "mask": mask,
            "maskT": maskT,
            "maskF": maskF,
            "maskF8": maskF8,
        })

    res = bass_utils.run_bass_kernel_spmd(nc, in_maps, list(range(N_CORES)))

    out = np.empty((B, T, D), dtype=np.float32)
    for c in range(N_CORES):
        b, h = c // 2, c % 2
        out[b, h * TC:(h + 1) * TC, :] = res.results[c]["out"]
    return out


# revision 21
# speedup vs baseline: 1.4611x; 1.2801x over previous
"""DepthAttentionResidual Trainium2 kernel.

Computation (see reference):
    ms      = mean(history^2, axis=-1)                      # [S,B,T]
    logits  = dot(query*rms_weight, history) * rsqrt(ms+eps)
    w       = softmax(logits, axis=S)
    out     = sum_s w[s] * history[s]                        # [B,T,D]

Sharding: data-parallel over (B=4) x (T halves) = 8 cores. Each core gets
hist [S=16, Tc=1024, D=1024] (64 MiB) and produces out [1024, 1024].

Per-core layout: partition p = s*8 + t' (S=16 depths x 8 t-blocks), D on
the free axis. A supertile is 128 t; slice g holds t_local = t'*16 + g,
so one slice is [128 partitions, 1024] and a supertile loads with
full-width 4 KiB-per-partition DMA descriptors (the 128-partition SBUF
port rule makes this ~6x faster than narrow-partition DMAs).
  - sum(h^2) over D: ScalarE activation(Square, accum_out) (last slice
    on VectorE to balance engines)
  - dot(q*w, h) over D: VectorE affine_mul_reduce (tensor_tensor_reduce
    crashes this runtime)
  - softmax over S: sum over s-partition-subgroups via 0/1-mask fp32
    matmuls (exact); exp on ScalarE; reciprocals on VectorE
  - depth mix: per D-half, 16 accumulating fp32r matmuls (1 cycle/row)
    with block-expanded masked weights built on GpSimd (w2[p, c] =
    softmax weight iff c == t_local(p, g)), filling all 128 t rows of a
    PSUM bank; PSUM -> SBUF on VectorE; one 512 KiB DMA per supertile.
Input DMAs ride the SP HWDGE ring; constants + output DMAs ride the
ScalarE ring so the history stream never stalls. The last supertile is
split 64/64 to shorten the serial tail. fp32r rounds operands to ~13
mantissa bits -> ~2e-4 relative output error.

Reads history exactly once (~68 MiB DMA per core): DMA-bound at ~305 us
vs a ~235 us descriptor-rate floor.
"""
import numpy as np

import concourse.bass as bass
import concourse.bacc as bacc
import concourse.tile as tile
from concourse import mybir
from concourse import bass_utils

N_CORES = 8
S = 16
B = 4
T = 2048
D = 1024
EPS = 1e-5

TC = T // 2          # t positions per core
TG = 8               # t-blocks per partition set (S * TG = 128 partitions)
GROUPS = 16          # stat slices per supertile (one t per partition each)
J = 1                # consecutive t rows per DMA slice (descriptor = J*4KiB)
NDMA = GROUPS // J   # input DMAs per supertile
TS = TG * GROUPS     # t per supertile = 128
N_SUPER = TC // TS   # supertiles per core = 8
F32 = mybir.dt.float32
I32 = mybir.dt.int32
F32R = mybir.dt.float32r
RSQRT_MAGIC = 0x5F3759DF


def _build_program():
    nc = bacc.Bacc("TRN2", target_bir_lowering=False, debug=False,
                   enable_asserts=True, num_devices=N_CORES)

    hist = nc.dram_tensor("hist", [S, TC, D], F32R, kind="ExternalInput").ap()
    query = nc.dram_tensor("query", [D], F32, kind="ExternalInput").ap()
    rmsw = nc.dram_tensor("rms_weight", [D], F32, kind="ExternalInput").ap()
    mask_d = nc.dram_tensor("mask", [128, TG], F32, kind="ExternalInput").ap()
    maskt_d = nc.dram_tensor("maskT", [TG, 128], F32, kind="ExternalInput").ap()
    maskf_d = nc.dram_tensor("maskF", [128, GROUPS, 128], F32,
                             kind="ExternalInput").ap()
    maskf8_d = nc.dram_tensor("maskF8", [128, GROUPS // 2, 128 // 2], F32,
                              kind="ExternalInput").ap()
    out = nc.dram_tensor("out", [TC, D], F32, kind="ExternalOutput").ap()

    with tile.TileContext(nc) as tc:
        with (
            tc.tile_pool(name="singles", bufs=1) as singles,
            tc.tile_pool(name="hsup", bufs=2) as hpool,
            tc.tile_pool(name="stats", bufs=2) as stats,
            tc.tile_pool(name="w2", bufs=3) as w2pool,
            tc.tile_pool(name="outp", bufs=2) as outpool,
            tc.tile_pool(name="ps_stats", bufs=2, space="PSUM") as ps_stats,
            tc.tile_pool(name="ps_mix", bufs=2, space="PSUM") as ps_mix,
        ):
            # ---- constants (DMAs emitted after the first supertile's so
            # the history stream starts immediately) ------------------------
            qw = singles.tile([128, D], F32)
            q1 = singles.tile([1, D], F32)
            w1 = singles.tile([1, D], F32)
            ones = singles.tile([1, 128], F32)
            mask = singles.tile([128, TG], F32)
            maskT = singles.tile([TG, 128], F32)
            maskF = singles.tile([128, GROUPS, 128], F32)
            maskF8 = singles.tile([128, GROUPS // 2, 128 // 2], F32)
            dummy_a = singles.tile([128, 1], F32)
            dummy_v = singles.tile([128, 1], F32)

            def emit_init():
                # constants ride the ScalarE HWDGE queue so they don't
                # delay the history stream on the SP queue. query and
                # rms_weight land on one partition (2 packets instead of a
                # 256-packet broadcast DMA) and are broadcast on-chip via a
                # 1-partition ones matmul.
                nc.scalar.dma_start(
                    out=q1[:],
                    in_=bass.AP(tensor=query.tensor, offset=0,
                                ap=[[0, 1], [1, D]]),
                )
                nc.scalar.dma_start(
                    out=w1[:],
                    in_=bass.AP(tensor=rmsw.tensor, offset=0,
                                ap=[[0, 1], [1, D]]),
                )
                nc.vector.memset(ones[:], 1.0)
                nc.vector.tensor_mul(q1[:], q1[:], w1[:])  # query * rms_weight
                for c in range(2):
                    bc = ps_mix.tile([128, 512], F32, tag="m", name=f"bc{c}")
                    nc.tensor.matmul(out=bc[:], lhsT=ones[:],
                                     rhs=q1[:, c * 512:(c + 1) * 512],
                                     start=True, stop=True)
                    nc.scalar.activation(
                        out=qw[:, c * 512:(c + 1) * 512], in_=bc[:],
                        func=mybir.ActivationFunctionType.Copy,
                    )
                nc.scalar.dma_start(out=mask[:], in_=mask_d)
                nc.scalar.dma_start(out=maskT[:], in_=maskt_d)
                nc.scalar.dma_start(out=maskF[:], in_=maskf_d)
                nc.scalar.dma_start(out=maskF8[:], in_=maskf8_d)

            # ---- main loop over supertiles --------------------------------
            # last 128-t supertile is split in two 64-t halves to shorten
            # the serial tail after the final DMA
            emit_init()
            schedule = [(k * TS, GROUPS) for k in range(N_SUPER - 1)]
            schedule += [((N_SUPER - 1) * TS, GROUPS // 2),
                         ((N_SUPER - 1) * TS + TS // 2, GROUPS // 2)]
            for k, (t0, groups) in enumerate(schedule):
                ts_k = TG * groups
                ndma = groups // J

                # load [S, 128t, D] as partitions (s, t') x free (g, d)
                # where t_local = t' * GROUPS + g: one full-width DMA with
                # 64 KiB contiguous per partition (128-partition P1 rule)
                half = max(ndma // 2, 1)
                hsupA = hpool.tile([128, half, J, D], F32R, tag="hsupA",
                                   name="hsupA", bufs=3)
                hsupB = hpool.tile([128, ndma - half, J, D], F32R,
                                   tag="hsupB", name="hsupB")

                def hslice(g):
                    gd, j = g // J, g % J
                    tile_ = hsupA if gd < half else hsupB
                    return tile_[:, gd - half if gd >= half else gd, j, :]

                srcv = hist[:, t0:t0 + ts_k, :].rearrange(
                    "s (t gd j) d -> s t gd (j d)", t=TG, gd=ndma)
                for gd in range(ndma):
                    tile_ = hsupA if gd < half else hsupB
                    nc.sync.dma_start(
                        out=tile_[:, gd - half if gd >= half else gd, :, :]
                        .rearrange("p j d -> p (j d)"),
                        in_=srcv[:, :, gd, :])


                ss = stats.tile([128, groups], F32, tag="ss")
                dot = stats.tile([128, groups], F32, tag="dot")
                for g in range(groups):
                    h_g = hslice(g).bitcast(F32)
                    if g < groups - 1:
                        nc.scalar.activation(
                            out=dummy_a.broadcast_to([128, D]),
                            in_=h_g,
                            func=mybir.ActivationFunctionType.Square,
                            accum_out=ss[:, g:g + 1],
                        )
                    else:
                        # last two sumsq on VectorE: keeps ScalarE below the
                        # DMA pace
                        nc.vector.affine_mul_reduce(
                            out=dummy_v.broadcast_to([128, D]),
                            accum_out=ss[:, g:g + 1],
                            in0=h_g, in1=h_g, scale=1.0, bias=0.0,
                        )
                    nc.vector.affine_mul_reduce(
                        out=dummy_v.broadcast_to([128, D]),
                        accum_out=dot[:, g:g + 1],
                        in0=h_g,
                        in1=qw[:],
                        scale=1.0,
                        bias=0.0,
                    )

                # rstd = rsqrt(ss/D + eps) via the int32 magic-constant
                # seed + 2 Newton steps, entirely on VectorE. The Sqrt
                # activation lives in a different table than Square/Exp and
                # cost 2 x 1.28 us of ACT_TABLE_LOAD per supertile; this
                # keeps ScalarE on the exp_and_others table for the whole
                # kernel (~5e-6 rel err, far under the fp32r mix noise).
                v = stats.tile([128, groups], F32, tag="v")
                nc.vector.tensor_scalar(
                    out=v[:], in0=ss[:], scalar1=1.0 / D, scalar2=EPS,
                    op0=mybir.AluOpType.mult, op1=mybir.AluOpType.add)
                rstd = stats.tile([128, groups], F32, tag="rstd")
                nc.vector.tensor_scalar(
                    out=rstd[:].bitcast(I32), in0=v[:].bitcast(I32),
                    scalar1=1, scalar2=None,
                    op0=mybir.AluOpType.logical_shift_right)
                nc.vector.tensor_scalar(
                    out=rstd[:].bitcast(I32), in0=rstd[:].bitcast(I32),
                    scalar1=-1, scalar2=RSQRT_MAGIC,
                    op0=mybir.AluOpType.mult, op1=mybir.AluOpType.add)
                t1 = stats.tile([128, groups], F32, tag="t1")
                t2 = stats.tile([128, groups], F32, tag="t2")
                for _ in range(2):  # y *= 1.5 - 0.5*v*y^2
                    nc.vector.tensor_mul(t1[:], rstd[:], rstd[:])
                    nc.vector.tensor_mul(t2[:], v[:], t1[:])
                    nc.vector.tensor_scalar(
                        out=t2[:], in0=t2[:], scalar1=-0.5, scalar2=1.5,
                        op0=mybir.AluOpType.mult, op1=mybir.AluOpType.add)
                    nc.vector.tensor_mul(rstd[:], rstd[:], t2[:])
                logit = stats.tile([128, groups], F32, tag="logit")
                nc.vector.tensor_mul(logit[:], dot[:], rstd[:])
                e = stats.tile([128, groups], F32, tag="e")
                nc.scalar.activation(
                    out=e[:], in_=logit[:],
                    func=mybir.ActivationFunctionType.Exp,
                )

                # sumexp over s: [8t', G] = mask^T @ e (exact fp32 matmul)
                se_ps = ps_stats.tile([TG, groups], F32, tag="se")
                nc.tensor.matmul(out=se_ps[:], lhsT=mask[:], rhs=e[:],
                                 start=True, stop=True)
                rse = stats.tile([TG, groups], F32, tag="rse")
                nc.vector.reciprocal(out=rse[:], in_=se_ps[:])
                # broadcast rse back to (s,t') partitions: maskT^T @ rse
                rseb_ps = ps_stats.tile([128, groups], F32, tag="rseb")
                nc.tensor.matmul(out=rseb_ps[:], lhsT=maskT[:], rhs=rse[:],
                                 start=True, stop=True)
                rseb = stats.tile([128, groups], F32, tag="rsebs")
                nc.vector.tensor_copy(out=rseb[:], in_=rseb_ps[:])

                # depth mix: accumulate 16 masked-weight matmuls per D chunk
                m_ps = [ps_mix.tile([TG * groups, 512], F32, tag="m", name=f"m{c}")
                        for c in range(2)]
                for g in range(groups):
                    w2 = w2pool.tile([128, TG * groups], F32R, tag="w2")
                    nc.gpsimd.tensor_scalar(
                        out=w2[:],
                        in0=(maskF[:, g, :] if groups == GROUPS
                             else maskF8[:, g, :]),
                        scalar1=e[:, g:g + 1],
                        scalar2=rseb[:, g:g + 1],
                        op0=mybir.AluOpType.mult,
                        op1=mybir.AluOpType.mult,
                    )
                    for c in range(2):
                        nc.tensor.matmul(
                            out=m_ps[c][:],
                            lhsT=w2[:],
                            rhs=hslice(g)[:, c * 512:(c + 1) * 512],
                            start=(g == 0),
                            stop=(g == groups - 1),
                        )

                ot = outpool.tile([TG * groups, D], F32, tag="ot")
                for c in range(2):
                    nc.vector.tensor_copy(out=ot[:, c * 512:(c + 1) * 512],
                                          in_=m_ps[c][:])
                nc.scalar.dma_start(out=out[t0:t0 + ts_k, :], in_=ot[:])

    nc.compile()
    return nc


_NC = None


def _get_program():
    global _NC
    if _NC is None:
        _NC = _build_program()
    return _NC


def _make_masks():
    # partition p = s*TG + t'; group slice g holds t_local = t'*GROUPS + g
    p = np.arange(128)
    mask = (p[:, None] % TG == np.arange(TG)[None, :]).astype(np.float32)
    maskF = np.zeros((128, GROUPS, 128), np.float32)
    for g in range(GROUPS):
        maskF[p, g, (p % TG) * GROUPS + g] = 1.0
    maskF8 = np.zeros((128, GROUPS // 2, 64), np.float32)
    for g in range(GROUPS // 2):
        maskF8[p, g, (p % TG) * (GROUPS // 2) + g] = 1.0
    return mask, np.ascontiguousarray(mask.T), maskF, maskF8


def kernel(history, query, rms_weight):
    history = np.asarray(history, dtype=np.float32)
    query = np.asarray(query, dtype=np.float32)
    rms_weight = np.asarray(rms_weight, dtype=np.float32)
    assert history.shape == (S, B, T, D), history.shape

    nc = _get_program()
    mask, maskT, maskF, maskF8 = _make_masks()

    in_maps = []
    for c in range(N_CORES):
        b, h = c // 2, c % 2
        shard = np.ascontiguousarray(history[:, b, h * TC:(h + 1) * TC, :])
        in_maps.append({
            "hist": shard,
            "query": query,
            "rms_weight": rms_weight,
            "mask": mask,
            "maskT": maskT,
            "maskF": maskF,
            "maskF8": maskF8,
        })

    res = bass_utils.run_bass_kernel_spmd(nc, in_maps, list(range(N_CORES)))

    out = np.empty((B, T, D), dtype=np.float32)
    for c in range(N_CORES):
        b, h = c // 2, c % 2
        out[b, h * TC:(h + 1) * TC, :] = res.results[c]["out"]
    return out



# revision 22
# speedup vs baseline: 1.5520x; 1.0622x over previous
"""DepthAttentionResidual Trainium2 kernel.

Computation (see reference):
    ms      = mean(history^2, axis=-1)                      # [S,B,T]
    logits  = dot(query*rms_weight, history) * rsqrt(ms+eps)
    w       = softmax(logits, axis=S)
    out     = sum_s w[s] * history[s]                        # [B,T,D]

Sharding: data-parallel over (B=4) x (T halves) = 8 cores. Each core gets
hist [S=16, Tc=1024, D=1024] (64 MiB) and produces out [1024, 1024].

Per-core layout: partition p = s*8 + t' (S=16 depths x 8 t-blocks), D on
the free axis. A supertile is 128 t; slice g holds t_local = t'*16 + g,
so one slice is [128 partitions, 1024] and a supertile loads with
full-width 4 KiB-per-partition DMA descriptors (the 128-partition SBUF
port rule makes this ~6x faster than narrow-partition DMAs).
  - sum(h^2) over D: ScalarE activation(Square, accum_out) (last slice
    on VectorE to balance engines)
  - dot(q*w, h) over D: VectorE affine_mul_reduce (tensor_tensor_reduce
    crashes this runtime)
  - softmax over S: sum over s-partition-subgroups via 0/1-mask fp32
    matmuls (exact); exp on ScalarE; reciprocals on VectorE
  - depth mix: per D-half, 16 accumulating fp32r matmuls (1 cycle/row)
    with block-expanded masked weights built on GpSimd (w2[p, c] =
    softmax weight iff c == t_local(p, g)), filling all 128 t rows of a
    PSUM bank; PSUM -> SBUF on VectorE; one 512 KiB DMA per supertile.
Input DMAs ride the SP HWDGE ring; constants + output DMAs ride the
ScalarE ring so the history stream never stalls. The last supertile is
split 64/64 to shorten the serial tail. fp32r rounds operands to ~13
mantissa bits -> ~2e-4 relative output error.

Reads history exactly once (~68 MiB DMA per core): DMA-bound at ~305 us
vs a ~235 us descriptor-rate floor.
"""
import numpy as np

import concourse.bass as bass
import concourse.bacc as bacc
import concourse.tile as tile
from concourse import mybir
from concourse import bass_utils

N_CORES = 8
S = 16
B = 4
T = 2048
D = 1024
EPS = 1e-5

TC = T // 2          # t positions per core
TG = 8               # t-blocks per partition set (S * TG = 128 partitions)
GROUPS = 16          # stat slices per supertile (one t per partition each)
J = 1                # consecutive t rows per DMA slice (descriptor = J*4KiB)
NDMA = GROUPS // J   # input DMAs per supertile
TS = TG * GROUPS     # t per supertile = 128
N_SUPER = TC // TS   # supertiles per core = 8
F32 = mybir.dt.float32
F32R = mybir.dt.float32r


def _build_program():
    nc = bacc.Bacc("TRN2", target_bir_lowering=False, debug=False,
                   enable_asserts=True, num_devices=N_CORES)

    hist = nc.dram_tensor("hist", [S, TC, D], F32R, kind="ExternalInput").ap()
    query = nc.dram_tensor("query", [D], F32, kind="ExternalInput").ap()
    rmsw = nc.dram_tensor("rms_weight", [D], F32, kind="ExternalInput").ap()
    mask_d = nc.dram_tensor("mask", [128, TG], F32, kind="ExternalInput").ap()
    maskt_d = nc.dram_tensor("maskT", [TG, 128], F32, kind="ExternalInput").ap()
    maskf_d = nc.dram_tensor("maskF", [128, GROUPS, 128], F32,
                             kind="ExternalInput").ap()
    maskf8_d = nc.dram_tensor("maskF8", [128, GROUPS // 2, 128 // 2], F32,
                              kind="ExternalInput").ap()
    out = nc.dram_tensor("out", [TC, D], F32, kind="ExternalOutput").ap()

    with tile.TileContext(nc) as tc:
        with (
            tc.tile_pool(name="singles", bufs=1) as singles,
            tc.tile_pool(name="hsup", bufs=2) as hpool,
            tc.tile_pool(name="stats", bufs=2) as stats,
            tc.tile_pool(name="w2", bufs=3) as w2pool,
            tc.tile_pool(name="outp", bufs=2) as outpool,
            tc.tile_pool(name="ps_stats", bufs=2, space="PSUM") as ps_stats,
            tc.tile_pool(name="ps_mix", bufs=2, space="PSUM") as ps_mix,
        ):
            # ---- constants (DMAs emitted after the first supertile's so
            # the history stream starts immediately) ------------------------
            qw = singles.tile([128, D], F32)
            wb = singles.tile([128, D], F32)
            mask = singles.tile([128, TG], F32)
            maskT = singles.tile([TG, 128], F32)
            maskF = singles.tile([128, GROUPS, 128], F32)
            maskF8 = singles.tile([128, GROUPS // 2, 128 // 2], F32)
            epst = singles.tile([128, 1], F32)
            dummy_a = singles.tile([128, 1], F32)
            dummy_v = singles.tile([128, 1], F32)

            def emit_init():
                # constants ride the ScalarE HWDGE queue so they don't
                # delay the history stream on the SP queue
                nc.scalar.dma_start(
                    out=qw[:],
                    in_=bass.AP(tensor=query.tensor, offset=0,
                                ap=[[0, 128], [1, D]]),
                )
                nc.scalar.dma_start(
                    out=wb[:],
                    in_=bass.AP(tensor=rmsw.tensor, offset=0,
                                ap=[[0, 128], [1, D]]),
                )
                nc.vector.tensor_mul(qw[:], qw[:], wb[:])  # query * rms_weight
                nc.scalar.dma_start(out=mask[:], in_=mask_d)
                nc.scalar.dma_start(out=maskT[:], in_=maskt_d)
                nc.scalar.dma_start(out=maskF[:], in_=maskf_d)
                nc.scalar.dma_start(out=maskF8[:], in_=maskf8_d)
                nc.vector.memset(epst[:], EPS)

            # ---- main loop over supertiles --------------------------------
            # last 128-t supertile is split in two 64-t halves to shorten
            # the serial tail after the final DMA
            emit_init()
            schedule = [(k * TS, GROUPS) for k in range(N_SUPER - 1)]
            schedule += [((N_SUPER - 1) * TS, GROUPS // 2),
                         ((N_SUPER - 1) * TS + TS // 2, GROUPS // 2)]
            for k, (t0, groups) in enumerate(schedule):
                ts_k = TG * groups
                ndma = groups // J

                # load [S, 128t, D] as partitions (s, t') x free (g, d)
                # where t_local = t' * GROUPS + g: one full-width DMA with
                # 64 KiB contiguous per partition (128-partition P1 rule)
                half = max(ndma // 2, 1)
                hsupA = hpool.tile([128, half, J, D], F32R, tag="hsupA",
                                   name="hsupA", bufs=3)
                hsupB = hpool.tile([128, ndma - half, J, D], F32R,
                                   tag="hsupB", name="hsupB")

                def hslice(g):
                    gd, j = g // J, g % J
                    tile_ = hsupA if gd < half else hsupB
                    return tile_[:, gd - half if gd >= half else gd, j, :]

                srcv = hist[:, t0:t0 + ts_k, :].rearrange(
                    "s (t gd j) d -> s t gd (j d)", t=TG, gd=ndma)
                for gd in range(ndma):
                    tile_ = hsupA if gd < half else hsupB
                    nc.sync.dma_start(
                        out=tile_[:, gd - half if gd >= half else gd, :, :]
                        .rearrange("p j d -> p (j d)"),
                        in_=srcv[:, :, gd, :])


                ss = stats.tile([128, groups], F32, tag="ss")
                dot = stats.tile([128, groups], F32, tag="dot")
                for g in range(groups):
                    h_g = hslice(g).bitcast(F32)
                    if g < groups - 1:
                        nc.scalar.activation(
                            out=dummy_a.broadcast_to([128, D]),
                            in_=h_g,
                            func=mybir.ActivationFunctionType.Square,
                            accum_out=ss[:, g:g + 1],
                        )
                    else:
                        # last two sumsq on VectorE: keeps ScalarE below the
                        # DMA pace
                        nc.vector.affine_mul_reduce(
                            out=dummy_v.broadcast_to([128, D]),
                            accum_out=ss[:, g:g + 1],
                            in0=h_g, in1=h_g, scale=1.0, bias=0.0,
                        )
                    nc.vector.affine_mul_reduce(
                        out=dummy_v.broadcast_to([128, D]),
                        accum_out=dot[:, g:g + 1],
                        in0=h_g,
                        in1=qw[:],
                        scale=1.0,
                        bias=0.0,
                    )

                # rstd = 1/sqrt(ss/D + eps); logits = dot * rstd; e = exp
                sd = stats.tile([128, groups], F32, tag="sd")
                nc.scalar.activation(
                    out=sd[:], in_=ss[:],
                    func=mybir.ActivationFunctionType.Sqrt,
                    bias=epst[:], scale=1.0 / D,
                )
                rstd = stats.tile([128, groups], F32, tag="rstd")
                nc.vector.reciprocal(out=rstd[:], in_=sd[:])
                logit = stats.tile([128, groups], F32, tag="logit")
                nc.vector.tensor_mul(logit[:], dot[:], rstd[:])
                e = stats.tile([128, groups], F32, tag="e")
                nc.scalar.activation(
                    out=e[:], in_=logit[:],
                    func=mybir.ActivationFunctionType.Exp,
                )

                # sumexp over s: [8t', G] = mask^T @ e (exact fp32 matmul)
                se_ps = ps_stats.tile([TG, groups], F32, tag="se")
                nc.tensor.matmul(out=se_ps[:], lhsT=mask[:], rhs=e[:],
                                 start=True, stop=True)
                rse = stats.tile([TG, groups], F32, tag="rse")
                nc.vector.reciprocal(out=rse[:], in_=se_ps[:])
                # broadcast rse back to (s,t') partitions: maskT^T @ rse
                rseb_ps = ps_stats.tile([128, groups], F32, tag="rseb")
                nc.tensor.matmul(out=rseb_ps[:], lhsT=maskT[:], rhs=rse[:],
                                 start=True, stop=True)
                rseb = stats.tile([128, groups], F32, tag="rsebs")
                nc.vector.tensor_copy(out=rseb[:], in_=rseb_ps[:])

                # depth mix: accumulate 16 masked-weight matmuls per D chunk
                m_ps = [ps_mix.tile([TG * groups, 512], F32, tag="m", name=f"m{c}")
                        for c in range(2)]
                for g in range(groups):
                    w2 = w2pool.tile([128, TG * groups], F32R, tag="w2")
                    nc.gpsimd.tensor_scalar(
                        out=w2[:],
                        in0=(maskF[:, g, :] if groups == GROUPS
                             else maskF8[:, g, :]),
                        scalar1=e[:, g:g + 1],
                        scalar2=rseb[:, g:g + 1],
                        op0=mybir.AluOpType.mult,
                        op1=mybir.AluOpType.mult,
                    )
                    for c in range(2):
                        nc.tensor.matmul(
                            out=m_ps[c][:],
                            lhsT=w2[:],
                            rhs=hslice(g)[:, c * 512:(c + 1) * 512],
                            start=(g == 0),
                            stop=(g == groups - 1),
                        )

                ot = outpool.tile([TG * groups, D], F32, tag="ot")
                for c in range(2):
                    nc.vector.tensor_copy(out=ot[:, c * 512:(c + 1) * 512],
                                          in_=m_ps[c][:])
                nc.scalar.dma_start(out=out[t0:t0 + ts_k, :], in_=ot[:])

    nc.compile()
    return nc


_NC = None


def _get_program():
    global _NC
    if _NC is None:
        _NC = _build_program()
    return _NC


def _make_masks():
    # partition p = s*TG + t'; group slice g holds t_local = t'*GROUPS + g
    p = np.arange(128)
    mask = (p[:, None] % TG == np.arange(TG)[None, :]).astype(np.float32)
    maskF = np.zeros((128, GROUPS, 128), np.float32)
    for g in range(GROUPS):
        maskF[p, g, (p % TG) * GROUPS + g] = 1.0
    maskF8 = np.zeros((128, GROUPS // 2, 64), np.float32)
    for g in range(GROUPS // 2):
        maskF8[p, g, (p % TG) * (GROUPS // 2) + g] = 1.0
    return mask, np.ascontiguousarray(mask.T), maskF, maskF8


def kernel(history, query, rms_weight):
    history = np.asarray(history, dtype=np.float32)
    query = np.asarray(query, dtype=np.float32)
    rms_weight = np.asarray(rms_weight, dtype=np.float32)
    assert history.shape == (S, B, T, D), history.shape

    nc = _get_program()
    mask, maskT, maskF, maskF8 = _make_masks()

    in_maps = []
    for c in range(N_CORES):
        b, h = c // 2, c % 2
        shard = np.ascontiguousarray(history[:, b, h * TC:(h + 1) * TC, :])
        in_maps.append({
            "hist": shard,
            "query": query,
            "rms_weight": rms_weight,
            "mask": mask,
            "maskT": maskT,
            "maskF": maskF,
            "maskF8": maskF8,
        })

    res = bass_utils.run_bass_kernel_spmd(nc, in_maps, list(range(N_CORES)))

    out = np.empty((B, T, D), dtype=np.float32)
    for c in range(N_CORES):
        b, h = c // 2, c % 2
        out[b, h * TC:(h + 1) * TC, :] = res.results[c]["out"]
    return out

